# revision 8
# baseline (speedup 1.0000x reference)
"""Trainium2 Bass kernel for nn_BDHModel (scatter_memory).

Computes, for T tokens:
  raw  = projection[tokens]                  # [T, N] gather
  thr  = 20th largest per row; acts = raw >= thr   (binary, K=20 active)
  scan: pred = sigma @ x; tension_t = 1 - <pred,x>/(|pred||x|+1e-8);
        sigma += 0.01 * outer(x,x), clipped to [0,1]

Key algebraic identity used on device: sigma starts at 0 and each entry grows
by +0.01 per co-activation. The clip at 1.0 binds only if some neuron pair
co-activates >100 times; for K=20-sparse random activations over T=256 steps
the max co-activation count is ~20 (verified host-side; numpy fallback
otherwise). With clip never binding:

  sigma_t = 0.01 * X_{<t}^T X_{<t}        (X = binary acts [T,N])
  pred_t  = 0.01 * X_{<t}^T g_t,  g_t = X_{<t} x_t = G[:t, t],  G = X X^T
  <pred_t, x_t>  = 0.01 * sum_{s<t} G[s,t]^2
  |pred_t|^2     = 1e-4 * g_t^T G_{<t,<t} g_t = 1e-4 * sum_s L[s,t] (G L)[s,t]
  with L = strictly-"earlier" masked G. So the serial scan collapses into a
  few small matmuls on the token-gram matrix G [T,T].

Device pipeline (single-core program, replicated SPMD on 8 cores):
  1. dma_gather of the T projection rows (token ids baked at compile time;
     the int16 index limit is handled by splitting the vocab at 32768 and
     permuting tokens so low-vocab tokens occupy a slot prefix).
  2. Exact top-20 threshold per row: 3 rounds of DVE max8 + match_replace
     on a scratch copy; thr = 4th value of round 3 (= 20th largest).
  3. acts = (raw >= thr) as bf16; PE-transpose to neuron-major XT.
  4. G = XT^T XT (PE, bf16 exact: entries are ints <= 20).
  5. L = G * mask(tv_s < tv_t)  (tv = original time of each slot, an input).
  6. M = G @ L (PE); dot = colsum(L*L); pn2 = colsum(L*M); cnt = colsum(XT).
  7. tension = 1 - dot / (sqrt(pn2*cnt) + 1e-6)   [identical regrouping of
     the reference's 1 - 0.01*dot / (0.01*sqrt(pn2)*sqrt(cnt) + 1e-8)].
  8. DMA out [1, T]; host un-permutes slots back to time order.
"""

import os
import numpy as np

T, N, K = 256, 1024, 20
VOCAB, HALF = 50257, 32768
NCH = N // 128   # 8 neuron chunks
TCH = T // 128   # 2 token chunks

LAST_RESULT = None  # BassKernelResults of the most recent device run


def _numpy_fallback(projection, sigma, tokens, plasticity):
    """Exact step-by-step emulation of the reference (f32). Only used if the
    fast-path preconditions fail (never, for the reference input family)."""
    proj = np.asarray(projection, np.float32)
    raw = proj[np.asarray(tokens)]
    kth = np.partition(raw, N - K, axis=1)[:, N - K]
    acts = (raw >= kth[:, None]).astype(np.float32)
    sig = np.array(sigma, np.float32, copy=True)
    out = np.zeros(T, np.float32)
    for t in range(T):
        x = acts[t]
        pred = (sig @ x).astype(np.float32)
        pn2 = np.float32(np.dot(pred, pred))
        pn = np.sqrt(pn2 if pn2 > 0 else np.float32(1.0))
        xn = np.float32(np.sqrt(np.dot(x, x)))
        overlap = np.float32(np.dot(pred, x)) / (pn * xn + np.float32(1e-8))
        out[t] = np.float32(1.0) - overlap if pn2 > 0 else np.float32(1.0)
        if plasticity:
            sig = np.clip(sig + np.float32(0.01) * np.outer(x, x), 0.0, 1.0)
    return out


def _gather_plans(ptok, nlow):
    """Per 128-token chunk: list of (chunk, table_half, local_idx_array).
    Emitted in program order; a junk-prefixed high-table gather is always
    emitted before the low-table gather that overwrites its junk slots."""
    plans = []
    for c in range(TCH):
        lc = int(np.clip(nlow - 128 * c, 0, 128))
        hc = 128 - lc
        lo = ptok[128 * c: 128 * c + lc]
        hi = ptok[128 * c + lc: 128 * (c + 1)]
        if hc > 0:
            idxs = np.concatenate([np.zeros(lc, np.int64), hi - HALF])
            plans.append((c, 1, idxs))
        if lc > 0:
            plans.append((c, 0, lo))
    return plans


def _wrap_idxs(idxs):
    """dma_gather index layout: slot j -> row j%16, col j//16, replicated to
    128 partitions; 8 int16 columns per gather."""
    w = np.full((16, 8), -1, np.int16)
    for j, v in enumerate(idxs):
        w[j % 16, j // 16] = v
    return np.tile(w, (8, 1))


def _build(tokens_np):
    """Build the Bass module with token ids baked in. Returns (nc, in_map, perm)."""
    from contextlib import ExitStack
    import concourse.bacc as bacc
    import concourse.mybir as mybir
    import concourse.tile as tile
    from concourse import masks

    dt = mybir.dt
    Alu = mybir.AluOpType
    Act = mybir.ActivationFunctionType

    tok = np.asarray(tokens_np, np.int64)
    lows = np.where(tok < HALF)[0]
    highs = np.where(tok >= HALF)[0]
    perm = np.concatenate([lows, highs])      # slot -> original position
    ptok = tok[perm]
    nlow = len(lows)
    plans = _gather_plans(ptok, nlow)

    gidx_np = np.concatenate([_wrap_idxs(p[2]) for p in plans], axis=1)
    tv = perm.astype(np.float32)              # time value per slot
    tvb_np = np.tile(tv[None, :], (128, 1))                  # [128, T]
    tvp_np = tv.reshape(TCH, 128).T.copy()                   # [128, TCH]

    nc = bacc.Bacc("TRN2", target_bir_lowering=False, debug=False,
                   enable_asserts=False, num_devices=1)

    proj_d = nc.dram_tensor("proj", [VOCAB, N], dt.float32, kind="ExternalInput")
    gidx_d = nc.dram_tensor("gidx", list(gidx_np.shape), dt.int16, kind="ExternalInput")
    tvb_d = nc.dram_tensor("tvb", [128, T], dt.float32, kind="ExternalInput")
    tvp_d = nc.dram_tensor("tvp", [128, TCH], dt.float32, kind="ExternalInput")
    out_d = nc.dram_tensor("tens", [1, T], dt.float32, kind="ExternalOutput")

    with tile.TileContext(nc) as tc, ExitStack() as ctx:
        pool = ctx.enter_context(tc.tile_pool(name="main", bufs=1))
        ppt = ctx.enter_context(tc.tile_pool(name="ppt", bufs=2, space="PSUM"))
        pacc = ctx.enter_context(tc.tile_pool(name="pacc", bufs=1, space="PSUM"))

        raw = pool.tile([128, TCH * N], dt.float32, tag="raw")
        rawc = pool.tile([128, TCH * N], dt.float32, tag="rawc")
        gidx = pool.tile([128, gidx_np.shape[1]], dt.int16, tag="gidx")
        tvb = pool.tile([128, T], dt.float32, tag="tvb")
        tvp = pool.tile([128, TCH], dt.float32, tag="tvp")
        m8 = pool.tile([128, 24 * TCH], dt.float32, tag="m8")
        acts = pool.tile([128, TCH * N], dt.bfloat16, tag="acts")
        ident = pool.tile([128, 128], dt.bfloat16, tag="ident")
        xt = pool.tile([128, NCH * T], dt.bfloat16, tag="xt")
        gf = pool.tile([128, TCH * T], dt.float32, tag="gf")
        gb = pool.tile([128, TCH * T], dt.bfloat16, tag="gb")
        msk = pool.tile([128, TCH * T], dt.float32, tag="msk")
        lf = pool.tile([128, TCH * T], dt.float32, tag="lf")
        lb = pool.tile([128, TCH * T], dt.bfloat16, tag="lb")
        ll = pool.tile([128, TCH * T], dt.float32, tag="ll")
        pp = pool.tile([128, TCH * T], dt.float32, tag="pp")
        ones_f = pool.tile([128, 1], dt.float32, tag="ones_f")
        ones_b = pool.tile([128, 1], dt.bfloat16, tag="ones_b")
        cnt_sb = pool.tile([1, T], dt.float32, tag="cnt_sb")
        q_sb = pool.tile([1, T], dt.float32, tag="q_sb")
        r_sb = pool.tile([1, T], dt.float32, tag="r_sb")
        rec_sb = pool.tile([1, T], dt.float32, tag="rec_sb")
        prod_sb = pool.tile([1, T], dt.float32, tag="prod_sb")
        tens_sb = pool.tile([1, T], dt.float32, tag="tens_sb")

        # --- constants + small input DMAs ---
        nc.sync.dma_start(gidx[:], gidx_d.ap())
        nc.sync.dma_start(tvb[:], tvb_d.ap())
        nc.sync.dma_start(tvp[:], tvp_d.ap())
        nc.gpsimd.memset(ones_f[:], 1.0)
        nc.gpsimd.memset(ones_b[:], 1.0)
        masks.make_identity(nc, ident[:])

        # --- 1. gather the T projection rows ---
        raw3 = raw[:].rearrange("p (c n) -> p c n", n=N)
        proj_ap = proj_d.ap()
        for g, (c, half, idxs) in enumerate(plans):
            in_ap = proj_ap[HALF:, :] if half else proj_ap
            nc.gpsimd.dma_gather(
                out_ap=raw3[:, c: c + 1, :],
                in_ap=in_ap,
                idxs_ap=gidx[:, 8 * g: 8 * g + (len(idxs) + 15) // 16],
                num_idxs=len(idxs),
                num_idxs_reg=int(len(idxs)),
                elem_size=N,
            )

        # --- 2+3. per token-chunk: pristine copy, top-20 threshold, acts ---
        for c in range(TCH):
            rc = raw[:, c * N:(c + 1) * N]
            rcc = rawc[:, c * N:(c + 1) * N]
            nc.scalar.activation(rcc, rc, Act.Copy)
            m1 = m8[:, c * 24 + 0: c * 24 + 8]
            m2 = m8[:, c * 24 + 8: c * 24 + 16]
            m3 = m8[:, c * 24 + 16: c * 24 + 24]
            nc.vector.max(m1, rc)
            nc.vector.match_replace(rc, m1, rc, -1e30)
            nc.vector.max(m2, rc)
            nc.vector.match_replace(rc, m2, rc, -1e30)
            nc.vector.max(m3, rc)
            thr = m8[:, c * 24 + 19: c * 24 + 20]   # 4th of round 3 = 20th
            nc.vector.tensor_scalar(
                acts[:, c * N:(c + 1) * N], rcc, thr, None, Alu.is_ge)

        # --- 3b. PE transpose acts -> XT [neuron, token] (bf16) ---
        for cn in range(NCH):
            pt = ppt.tile([128, T], dt.bfloat16, tag="pt")
            for r in range(TCH):
                nc.tensor.transpose(
                    pt[:, r * 128:(r + 1) * 128],
                    acts[:, r * N + cn * 128: r * N + (cn + 1) * 128],
                    ident[:],
                )
            nc.scalar.activation(xt[:, cn * T:(cn + 1) * T], pt[:], Act.Copy)

        # --- 4. G = X X^T  [T, T] f32 psum, via bf16 matmuls (exact) ---
        gps = []
        for m in range(TCH):
            gp = pacc.tile([128, T], dt.float32, tag="big")
            for cn in range(NCH):
                nc.tensor.matmul(
                    gp[:],
                    xt[:, cn * T + m * 128: cn * T + (m + 1) * 128],
                    xt[:, cn * T:(cn + 1) * T],
                    start=(cn == 0), stop=(cn == NCH - 1),
                )
            gps.append(gp)
        cntp = pacc.tile([1, T], dt.float32, tag="cntp")
        for cn in range(NCH):
            nc.tensor.matmul(cntp[:], ones_b[:], xt[:, cn * T:(cn + 1) * T],
                             start=(cn == 0), stop=(cn == NCH - 1))

        # --- 5. masks and L ---
        for m in range(TCH):
            gfm = gf[:, m * T:(m + 1) * T]
            gbm = gb[:, m * T:(m + 1) * T]
            nc.scalar.activation(gfm, gps[m][:], Act.Copy)
            nc.vector.tensor_copy(gbm, gps[m][:])
            mm = msk[:, m * T:(m + 1) * T]
            nc.vector.tensor_scalar(mm, tvb[:], tvp[:, m: m + 1], None, Alu.is_gt)
            lfm = lf[:, m * T:(m + 1) * T]
            nc.vector.tensor_mul(lfm, gfm, mm)
            nc.scalar.activation(lb[:, m * T:(m + 1) * T], lfm, Act.Copy)

        # --- 6. M = G @ L (psum), dot/pn2/cnt colsums ---
        mps = []
        for m in range(TCH):
            mp = pacc.tile([128, T], dt.float32, tag="big")
            for b in range(TCH):
                nc.tensor.matmul(
                    mp[:],
                    gb[:, b * T + m * 128: b * T + (m + 1) * 128],
                    lb[:, b * T:(b + 1) * T],
                    start=(b == 0), stop=(b == TCH - 1),
                )
            mps.append(mp)
        stats = pacc.tile([1, 2 * T], dt.float32, tag="stats")
        dotp = stats[:, 0:T]
        pn2p = stats[:, T:2 * T]
        for m in range(TCH):
            lfm = lf[:, m * T:(m + 1) * T]
            nc.vector.tensor_mul(ll[:, m * T:(m + 1) * T], lfm, lfm)
            nc.vector.tensor_mul(pp[:, m * T:(m + 1) * T], lfm, mps[m][:])
        for m in range(TCH):
            nc.tensor.matmul(dotp, ones_f[:], ll[:, m * T:(m + 1) * T],
                             start=(m == 0), stop=(m == TCH - 1))
        for m in range(TCH):
            nc.tensor.matmul(pn2p, ones_f[:], pp[:, m * T:(m + 1) * T],
                             start=(m == 0), stop=(m == TCH - 1))

        # --- 7. final per-token math on [1, T] ---
        nc.scalar.activation(cnt_sb[:], cntp[:], Act.Copy)
        nc.vector.tensor_mul(q_sb[:], pn2p, cnt_sb[:])
        nc.scalar.activation(r_sb[:], q_sb[:], Act.Sqrt)
        nc.vector.tensor_scalar_add(r_sb[:], r_sb[:], 1e-6)
        nc.vector.reciprocal(rec_sb[:], r_sb[:])
        nc.vector.tensor_mul(prod_sb[:], dotp, rec_sb[:])
        nc.scalar.activation(tens_sb[:], prod_sb[:], Act.Copy, bias=1.0, scale=-1.0)

        # --- 8. output ---
        nc.sync.dma_start(out_d.ap(), tens_sb[:])

    nc.compile()

    in_map = {
        "proj": None,  # filled by caller (f32 [VOCAB, N])
        "gidx": gidx_np,
        "tvb": tvb_np,
        "tvp": tvp_np,
    }
    return nc, in_map, perm


def _fast_path_ok(projection, sigma, tokens):
    """Host-side guard that the algebraic rewrite is exact for this input:
    sigma starts at zero and the 0.01-per-co-activation growth never reaches
    the clip at 1.0 (i.e. max pairwise co-activation count <= 100)."""
    if np.any(np.asarray(sigma)):
        return False
    proj = np.asarray(projection, np.float32)
    raw = proj[np.asarray(tokens)]
    kth = np.partition(raw, N - K, axis=1)[:, N - K]
    acts = (raw >= kth[:, None]).astype(np.float32)
    coact = acts.T @ acts
    return float(coact.max()) <= 100.0


def kernel(projection, sigma, tokens, plasticity):
    global LAST_RESULT
    projection = np.ascontiguousarray(np.asarray(projection, np.float32))
    sigma = np.asarray(sigma, np.float32)
    tokens = np.asarray(tokens).astype(np.int64)
    plast = int(np.asarray(plasticity).reshape(-1)[0]) if np.ndim(plasticity) or True else int(plasticity)

    if not plast:
        # sigma never updates; with sigma == 0, pred == 0 -> tension == 1.
        if not np.any(sigma):
            return np.ones(T, np.float32)
        return _numpy_fallback(projection, sigma, tokens, plast)
    if not _fast_path_ok(projection, sigma, tokens):
        return _numpy_fallback(projection, sigma, tokens, plast)

    from concourse.bass_utils import run_bass_kernel_spmd

    nc, in_map, perm = _build(tokens)
    in_map["proj"] = projection
    n_cores = int(os.environ.get("BDH_CORES", "8"))
    try:
        res = run_bass_kernel_spmd(
            nc,
            [dict(in_map) for _ in range(n_cores)],
            core_ids=list(range(n_cores)),
        )
    except ModuleNotFoundError:
        # BASS_TRACE was requested but this axon build has no NTFF hook.
        os.environ["BASS_NEVER_TRACE"] = "1"
        res = run_bass_kernel_spmd(
            nc,
            [dict(in_map) for _ in range(n_cores)],
            core_ids=list(range(n_cores)),
        )
    LAST_RESULT = res
    tens_slots = res.results[0]["tens"].reshape(T).astype(np.float32)
    out = np.empty(T, np.float32)
    out[perm] = tens_slots
    return out


# revision 39
# speedup vs baseline: 1.5409x; 1.5409x over previous
"""Trainium2 Bass kernel for nn_BDHModel (scatter_memory).

Computes, for T tokens:
  raw  = projection[tokens]                  # [T, N] gather
  thr  = 20th largest per row; acts = raw >= thr   (binary, K=20 active)
  scan: pred = sigma @ x; tension_t = 1 - <pred,x>/(|pred||x|+1e-8);
        sigma += 0.01 * outer(x,x), clipped to [0,1]

Key algebraic identity used on device: sigma starts at 0 and each entry grows
by +0.01 per co-activation. The clip at 1.0 binds only if some neuron pair
co-activates >100 times; for K=20-sparse random activations over T=256 steps
the max co-activation count is ~20 (verified host-side; numpy fallback
otherwise). With clip never binding:

  sigma_t = 0.01 * X_{<t}^T X_{<t}        (X = binary acts [T,N])
  pred_t  = 0.01 * X_{<t}^T g_t,  g_t = X_{<t} x_t = G[:t, t],  G = X X^T
  <pred_t, x_t>  = 0.01 * sum_{s<t} G[s,t]^2
  |pred_t|^2     = 1e-4 * g_t^T G_{<t,<t} g_t = 1e-4 * sum_s L[s,t] (G L)[s,t]
  with L = strictly-"earlier" masked G. So the serial scan collapses into a
  few small matmuls on the token-gram matrix G [T,T].

Device pipeline (single-core program, replicated SPMD on 8 cores):
  1. dma_gather of the T projection rows (token ids baked at compile time;
     the int16 index limit is handled by splitting the vocab at 32768 and
     permuting tokens so low-vocab tokens occupy a slot prefix; the one
     mixed 128-token chunk is assembled via a parallel scratch gather and a
     partition-aligned stitch copy).
  2. Exact top-20 threshold per 1024-wide row on the DVE:
     - segmented path (validity host-verified per input): top-8 of each of
       16 64-wide segments via max8, then 3 max8 + 2 match_replace merge
       rounds over the 128 candidates; thr = 4th value of round 3.
     - fallback: 3 max8 + 2 match_replace rounds over the full row.
  3. acts = (raw >= thr) as bf16 (with per-row count via accum_out);
     PE-transpose to neuron-major XT.
  4. G = XT^T XT (PE, bf16 exact: entries are ints <= 20).
  5. L = G * mask, mask[s,t] = [time(s) < time(t)] precomputed host-side
     (handles the vocab-split token permutation).
  6. M = G @ L (PE); dot = colsum(L*L); pn2 = colsum(L*M).
  7. tension = 1 - dot / (sqrt(pn2*cnt) + 1e-6)   [identical regrouping of
     the reference's 1 - 0.01*dot / (0.01*sqrt(pn2)*sqrt(cnt) + 1e-8)].
  8. DMA out [1, T]; host un-permutes slots back to time order.
"""

import os
import numpy as np

T, N, K = 256, 1024, 20
VOCAB, HALF = 50257, 32768
NCH = N // 128   # 8 neuron chunks
TCH = T // 128   # 2 token chunks

LAST_RESULT = None  # BassKernelResults of the most recent device run


def _numpy_fallback(projection, sigma, tokens, plasticity):
    """Exact step-by-step emulation of the reference (f32). Only used if the
    fast-path preconditions fail (never, for the reference input family)."""
    proj = np.asarray(projection, np.float32)
    raw = proj[np.asarray(tokens)]
    kth = np.partition(raw, N - K, axis=1)[:, N - K]
    acts = (raw >= kth[:, None]).astype(np.float32)
    sig = np.array(sigma, np.float32, copy=True)
    out = np.zeros(T, np.float32)
    for t in range(T):
        x = acts[t]
        pred = (sig @ x).astype(np.float32)
        pn2 = np.float32(np.dot(pred, pred))
        pn = np.sqrt(pn2 if pn2 > 0 else np.float32(1.0))
        xn = np.float32(np.sqrt(np.dot(x, x)))
        overlap = np.float32(np.dot(pred, x)) / (pn * xn + np.float32(1e-8))
        out[t] = np.float32(1.0) - overlap if pn2 > 0 else np.float32(1.0)
        if plasticity:
            sig = np.clip(sig + np.float32(0.01) * np.outer(x, x), 0.0, 1.0)
    return out


def _plan_gathers(ptok, nlow):
    """Returns (gathers, stitches). Each gather: (dest, chunk, half, idxs)
    with dest in {"raw", "scr"}; all gathers write disjoint tiles and run in
    parallel. Each stitch: (chunk, part_off, rows) — a partition-aligned ACT
    copy scr[part_off:part_off+rows] -> raw_chunk[part_off:...]."""
    gathers, stitches = [], []
    for c in range(TCH):
        lc = int(np.clip(nlow - 128 * c, 0, 128))
        hc = 128 - lc
        lo = ptok[128 * c: 128 * c + lc]
        hi = ptok[128 * c + lc: 128 * (c + 1)]
        if hc == 0:
            gathers.append(("raw", c, 0, lo))
        elif lc == 0:
            gathers.append(("raw", c, 1, hi - HALF))
        else:
            # lows go to scratch partitions [0, lc); highs go straight into
            # the chunk with an lc-row junk prefix (overwritten by the
            # stitch copy, which starts at partition 0 as engines require)
            gathers.append(("scr", c, 0, lo))
            idxs = np.concatenate([np.zeros(lc, np.int64), hi - HALF])
            gathers.append(("raw", c, 1, idxs))
            stitches.append((c, 0, lc))
    return gathers, stitches


def _wrap_idxs(idxs):
    """dma_gather index layout: slot j -> row j%16, col j//16, replicated to
    128 partitions; 8 int16 columns per gather."""
    w = np.full((16, 8), -1, np.int16)
    for j, v in enumerate(idxs):
        w[j % 16, j // 16] = v
    return np.tile(w, (8, 1))


def _build(tokens_np, nseg=16):
    """Build the Bass module with token ids baked in. Returns (nc, in_map, perm)."""
    from contextlib import ExitStack
    import concourse.bacc as bacc
    import concourse.mybir as mybir
    import concourse.tile as tile
    from concourse import masks
    from concourse.tile import add_dep_helper

    dt = mybir.dt
    Alu = mybir.AluOpType
    Act = mybir.ActivationFunctionType

    tok = np.asarray(tokens_np, np.int64)
    lows = np.where(tok < HALF)[0]
    highs = np.where(tok >= HALF)[0]
    perm = np.concatenate([lows, highs])      # slot -> original position
    ptok = tok[perm]
    nlow = len(lows)
    gathers, stitches = _plan_gathers(ptok, nlow)

    gidx_np = np.concatenate([_wrap_idxs(g[3]) for g in gathers], axis=1)
    tv = perm.astype(np.float32)              # original time per slot
    # msk[m][p, t]  = 1.0 iff time(128m+p) < time(t)   (L in [s, t] layout)
    # msk2[m][p, s] = 1.0 iff time(s) < time(128m+p)   (L^T in [t, s] layout)
    msk_np = np.concatenate(
        [(tv[None, :] > tv[128 * m: 128 * (m + 1), None]).astype(np.float32)
         for m in range(TCH)], axis=1)        # [128, TCH*T]
    msk2_np = np.concatenate(
        [(tv[None, :] < tv[128 * m: 128 * (m + 1), None]).astype(np.float32)
         for m in range(TCH)], axis=1)        # [128, TCH*T]

    nc = bacc.Bacc("TRN2", target_bir_lowering=False, debug=False,
                   enable_asserts=False, num_devices=1)

    proj_d = nc.dram_tensor("proj", [VOCAB, N], dt.float32, kind="ExternalInput")
    gidx_d = nc.dram_tensor("gidx", list(gidx_np.shape), dt.int16, kind="ExternalInput")
    msk_d = nc.dram_tensor("msk", [128, TCH * T], dt.float32, kind="ExternalInput")
    msk2_d = nc.dram_tensor("msk2", [128, TCH * T], dt.float32, kind="ExternalInput")
    out_d = nc.dram_tensor("tens", [128, TCH], dt.float32, kind="ExternalOutput")

    with tile.TileContext(nc) as tc, ExitStack() as ctx:
        pool = ctx.enter_context(tc.tile_pool(name="main", bufs=1))
        ppt = ctx.enter_context(tc.tile_pool(name="ppt", bufs=4, space="PSUM"))
        pacc = ctx.enter_context(tc.tile_pool(name="pacc", bufs=1, space="PSUM"))

        raw = pool.tile([128, TCH * N], dt.float32, tag="raw")
        scr = pool.tile([128, N], dt.float32, tag="scr")
        gidx = pool.tile([128, gidx_np.shape[1]], dt.int16, tag="gidx")
        msk = pool.tile([128, TCH * T], dt.float32, tag="msk")
        msk2 = pool.tile([128, TCH * T], dt.float32, tag="msk2")
        seg_topk = nseg > 0
        cand = pool.tile([128, 8 * max(nseg, 1) * TCH], dt.float32, tag="cand")
        rawc = None if seg_topk else pool.tile([128, TCH * N], dt.float32, tag="rawc")
        m8 = pool.tile([128, 24 * TCH], dt.float32, tag="m8")
        acts = pool.tile([128, TCH * N], dt.bfloat16, tag="acts")
        ident = pool.tile([128, 128], dt.bfloat16, tag="ident")
        xt = pool.tile([128, NCH * T], dt.bfloat16, tag="xt")
        gb = pool.tile([128, TCH * T], dt.bfloat16, tag="gb")
        lt = pool.tile([128, TCH * T], dt.float32, tag="lt")
        lb = pool.tile([128, TCH * T], dt.bfloat16, tag="lb")
        dump = pool.tile([128, T], dt.float32, tag="dump")
        prod1 = pool.tile([128, TCH * T], dt.float32, tag="prod1")
        prod2 = pool.tile([128, TCH * T], dt.float32, tag="prod2")
        cnt_pm = pool.tile([128, TCH], dt.float32, tag="cnt_pm")
        dotv = pool.tile([128, TCH], dt.float32, tag="dotv")
        pn2v = pool.tile([128, TCH], dt.float32, tag="pn2v")
        q_v = pool.tile([128, TCH], dt.float32, tag="q_v")
        r_v = pool.tile([128, TCH], dt.float32, tag="r_v")
        rec_v = pool.tile([128, TCH], dt.float32, tag="rec_v")
        prod_v = pool.tile([128, TCH], dt.float32, tag="prod_v")
        tens_v = pool.tile([128, TCH], dt.float32, tag="tens_v")
        pre_v = pool.tile([128, 1], dt.float32, tag="pre_v")

        # --- constants, ACT table preloads, small input DMAs ---
        nc.sync.dma_start(gidx[:], gidx_d.ap())
        nc.sync.dma_start(msk[:], msk_d.ap())
        nc.sync.dma_start(msk2[:], msk2_d.ap())
        # preload ACT function tables off the critical path (sqrt(1)=1)
        nc.gpsimd.memset(pre_v[:], 1.0)
        nc.scalar.activation(pre_v[:], pre_v[:], Act.Copy)
        nc.scalar.activation(pre_v[:], pre_v[:], Act.Sqrt)
        masks.make_identity(nc, ident[:])

        # --- 1. gathers (all parallel; disjoint dest tiles) + stitch ---
        raw3 = raw[:].rearrange("p (c n) -> p c n", n=N)
        scr3 = scr[:].rearrange("p (c n) -> p c n", n=N)
        proj_ap = proj_d.ap()
        for g, (dest, c, half, idxs) in enumerate(gathers):
            out_ap = raw3[:, c: c + 1, :] if dest == "raw" else scr3[:, 0:1, :]
            nc.gpsimd.dma_gather(
                out_ap=out_ap,
                in_ap=proj_ap[HALF:, :] if half else proj_ap,
                idxs_ap=gidx[:, 8 * g: 8 * g + (len(idxs) + 15) // 16],
                num_idxs=len(idxs),
                num_idxs_reg=int(len(idxs)),
                elem_size=N,
            )
        for c, off, rows in stitches:
            nc.scalar.activation(
                raw[off:off + rows, c * N:(c + 1) * N],
                scr[off:off + rows, :], Act.Copy)

        # --- 2+3. per token-chunk: top-20 threshold, acts (+ row counts) ---
        prev_last = None
        for c in range(TCH):
            rc = raw[:, c * N:(c + 1) * N]
            chunk_ops = []
            if seg_topk:
                segw = N // nseg
                cd = cand[:, c * 8 * nseg:(c + 1) * 8 * nseg]
                for s in range(nseg):
                    op = nc.vector.max(
                        cd[:, s * 8:(s + 1) * 8],
                        rc[:, s * segw:(s + 1) * segw])
                    chunk_ops.append(op)
                sel = cd
            else:
                op = nc.scalar.activation(rawc[:, c * N:(c + 1) * N], rc, Act.Copy)
                sel = rawc[:, c * N:(c + 1) * N]
                rc = sel
                chunk_ops.append(op)
            m1 = m8[:, c * 24 + 0: c * 24 + 8]
            m2 = m8[:, c * 24 + 8: c * 24 + 16]
            m3 = m8[:, c * 24 + 16: c * 24 + 24]
            src = sel if seg_topk else raw[:, c * N:(c + 1) * N]
            chunk_ops.append(nc.vector.max(m1, src))
            chunk_ops.append(nc.vector.match_replace(src, m1, src, -1e30))
            chunk_ops.append(nc.vector.max(m2, src))
            chunk_ops.append(nc.vector.match_replace(src, m2, src, -1e30))
            chunk_ops.append(nc.vector.max(m3, src))
            thr = m8[:, c * 24 + 19: c * 24 + 20]   # 4th of round 3 = 20th
            last = nc.vector.tensor_scalar(
                acts[:, c * N:(c + 1) * N], rc, thr, None, Alu.is_ge,
                Alu.add, accum_out=cnt_pm[:, c: c + 1])
            chunk_ops.append(last)
            # keep the DVE chain chunk-ordered so chunk 0 finishes early and
            # its transposes/G overlap chunk 1's top-k
            if prev_last is not None:
                for op in chunk_ops:
                    add_dep_helper(op.ins, prev_last.ins, sync=False,
                                   reason="chunk-order DVE chain")
            prev_last = last

        # --- 3b. PE transpose acts -> XT [neuron, token] (bf16) ---
        # blocks grouped by token-half r so all r=0 work (transpose, copy,
        # and the G half-matmuls below) overlaps chunk 1's top-k; four
        # 128x128 transposes pack into one PSUM tile so one wide copy
        # evacuates them. xt free layout: index = r*N + cn*128.
        for r in range(TCH):
            for g in range(NCH // 4):
                pt = ppt.tile([128, 512], dt.bfloat16, tag="pt")
                for j in range(4):
                    cn = g * 4 + j
                    nc.tensor.transpose(
                        pt[:, j * 128:(j + 1) * 128],
                        acts[:, r * N + cn * 128: r * N + (cn + 1) * 128],
                        ident[:],
                    )
                dst = xt[:, r * N + g * 512: r * N + (g + 1) * 512]
                if r == 0 or g % 2 == 0:
                    # ACT: the DVE must not be interrupted mid-top-k (r=0)
                    nc.scalar.activation(dst, pt[:], Act.Copy)
                else:
                    nc.vector.tensor_copy(dst, pt[:])

        # --- 4. G = X X^T  [T, T] f32 psum, via bf16 matmuls (exact),
        #        split by token-half r so the r=0 half runs early ---
        gps = []
        for m in range(TCH):
            gp = pacc.tile([128, T], dt.float32, tag=f"g{m}")
            gps.append(gp)
        for r in range(TCH):
            for m in range(TCH):
                for cn in range(NCH):
                    nc.tensor.matmul(
                        gps[m][:, r * 128:(r + 1) * 128],
                        xt[:, m * N + cn * 128: m * N + (cn + 1) * 128],
                        xt[:, r * N + cn * 128: r * N + (cn + 1) * 128],
                        start=(cn == 0), stop=(cn == NCH - 1),
                    )

        # --- 5+6. masked prefix matrices straight from PSUM, M^T = L^T G,
        #        and the dot/pn2 row reductions — all split by token-half so
        #        every piece gated only on r=0 data runs during chunk 1's
        #        top-k. Emission order == dependency order (r ascending).
        #        lb = bf16(G * msk)   (L, [s, t] layout — lhsT for M^T)
        #        lt = f32 (G * msk2)  (L^T, [t, s] layout — for row TTRs)
        #        gb = bf16(G)         (rhs for M^T) ---
        mts = []
        for m in range(TCH):
            mt = pacc.tile([128, T], dt.float32, tag=f"mt{m}")
            mts.append(mt)

        # gb halves by r (ACT — free during chunk 1's top-k); lb/lt as full
        # DVE ops (DVE is the serial resource; splitting only adds overhead)
        for r in range(TCH):
            for m in range(TCH):
                sl = slice(m * T + r * 128, m * T + (r + 1) * 128)
                nc.scalar.activation(gb[:, sl],
                                     gps[m][:, r * 128:(r + 1) * 128], Act.Copy)
        # lb first: it unblocks the M^T matmuls on the PE
        for m in range(TCH):
            nc.vector.tensor_mul(lb[:, m * T:(m + 1) * T], gps[m][:],
                                 msk[:, m * T:(m + 1) * T])
        # NOTE: tensor_tensor_reduce is rejected by this runtime (device
        # NRT_EXEC_UNIT_UNRECOVERABLE) — reductions use an exact DVE product
        # followed by an ACT Copy with accum_out (HW-verified) instead.
        for m in range(TCH):
            ltm = lt[:, m * T:(m + 1) * T]
            nc.vector.tensor_mul(ltm, gps[m][:], msk2[:, m * T:(m + 1) * T])
            # dot[t] = sum_s L^T[t,s]^2 — needs only lt, runs before M
            nc.vector.tensor_mul(prod1[:, m * T:(m + 1) * T], ltm, ltm)
            nc.scalar.activation(dump[:], prod1[:, m * T:(m + 1) * T],
                                 Act.Copy, accum_out=dotv[:, m: m + 1])
        for m in range(TCH):
            for b in range(TCH):
                nc.tensor.matmul(
                    mts[m][:],
                    lb[:, b * T + m * 128: b * T + (m + 1) * 128],
                    gb[:, b * T:(b + 1) * T],
                    start=(b == 0), stop=(b == TCH - 1),
                )
            nc.vector.tensor_mul(prod2[:, m * T:(m + 1) * T],
                                 lt[:, m * T:(m + 1) * T], mts[m][:])
            nc.scalar.activation(dump[:], prod2[:, m * T:(m + 1) * T],
                                 Act.Copy, accum_out=pn2v[:, m: m + 1])

        # --- 7. final per-token math on [128, TCH] (token-major):
        #     tension = 1 - dot/denom = (denom - dot)/denom,
        #     denom = sqrt(pn2*cnt) + 1e-6 ---
        nc.vector.tensor_mul(q_v[:], pn2v[:], cnt_pm[:])
        nc.scalar.activation(r_v[:], q_v[:], Act.Sqrt)
        nc.vector.tensor_scalar_add(r_v[:], r_v[:], 1e-6)
        nc.vector.tensor_tensor(prod_v[:], r_v[:], dotv[:], Alu.subtract)
        nc.vector.reciprocal(rec_v[:], r_v[:])
        nc.vector.tensor_mul(tens_v[:], prod_v[:], rec_v[:])

        # --- 8. output: plain [128, TCH] DMA; host maps (p, c) -> t = 128c+p ---
        nc.sync.dma_start(out_d.ap(), tens_v[:])

    nc.compile()

    in_map = {
        "proj": None,  # filled by caller (f32 [VOCAB, N])
        "gidx": gidx_np,
        "msk": msk_np,
        "msk2": msk2_np,
    }
    return nc, in_map, perm


def _check_input(projection, sigma, tokens):
    """Host-side guards. Returns (fast_ok, nseg):
    fast_ok — the algebraic rewrite is exact (sigma==0, clip never binds);
    nseg    — widest valid segmentation for the segmented top-k (a
    segmentation is valid when taking the top-8 of every segment still
    captures all of each row's top-20 values), or 0 for the full-row path."""
    if np.any(np.asarray(sigma)):
        return False, 0
    proj = np.asarray(projection, np.float32)
    raw = proj[np.asarray(tokens)]
    kth = np.partition(raw, N - K, axis=1)[:, N - K]
    acts = (raw >= kth[:, None]).astype(np.float32)
    coact = acts.T @ acts
    fast_ok = float(coact.max()) <= 100.0
    nseg = 0
    for cand_nseg in (8, 16):
        segs = raw.reshape(T, cand_nseg, N // cand_nseg)
        cand = -np.sort(-segs, axis=2)[:, :, :8].reshape(T, cand_nseg * 8)
        thr_dev = -np.sort(-cand, axis=1)[:, K - 1]
        if bool(np.all(thr_dev == kth)):
            nseg = cand_nseg
            break
    return fast_ok, nseg


def kernel(projection, sigma, tokens, plasticity):
    global LAST_RESULT
    projection = np.ascontiguousarray(np.asarray(projection, np.float32))
    sigma = np.asarray(sigma, np.float32)
    tokens = np.asarray(tokens).astype(np.int64)
    plast = int(np.asarray(plasticity).reshape(-1)[0]) if np.ndim(plasticity) else int(plasticity)

    if not plast:
        # sigma never updates; with sigma == 0, pred == 0 -> tension == 1.
        if not np.any(sigma):
            return np.ones(T, np.float32)
        return _numpy_fallback(projection, sigma, tokens, plast)
    fast_ok, nseg = _check_input(projection, sigma, tokens)
    if not fast_ok:
        return _numpy_fallback(projection, sigma, tokens, plast)

    from concourse.bass_utils import run_bass_kernel_spmd

    nc, in_map, perm = _build(tokens, nseg=nseg)
    in_map["proj"] = projection
    n_cores = int(os.environ.get("BDH_CORES", "8"))
    try:
        res = run_bass_kernel_spmd(
            nc,
            [dict(in_map) for _ in range(n_cores)],
            core_ids=list(range(n_cores)),
        )
    except ModuleNotFoundError:
        # BASS_TRACE was requested but this axon build has no NTFF hook.
        os.environ["BASS_NEVER_TRACE"] = "1"
        res = run_bass_kernel_spmd(
            nc,
            [dict(in_map) for _ in range(n_cores)],
            core_ids=list(range(n_cores)),
        )
    LAST_RESULT = res
    # device layout [p, c] -> slot t = 128c + p; then slot -> original time
    tens_slots = res.results[0]["tens"].reshape(128, TCH).T.reshape(T)
    out = np.empty(T, np.float32)
    out[perm] = tens_slots.astype(np.float32)
    return out


# revision 42
# speedup vs baseline: 1.5645x; 1.0154x over previous
"""Trainium2 Bass kernel for nn_BDHModel (scatter_memory).

Computes, for T tokens:
  raw  = projection[tokens]                  # [T, N] gather
  thr  = 20th largest per row; acts = raw >= thr   (binary, K=20 active)
  scan: pred = sigma @ x; tension_t = 1 - <pred,x>/(|pred||x|+1e-8);
        sigma += 0.01 * outer(x,x), clipped to [0,1]

Key algebraic identity used on device: sigma starts at 0 and each entry grows
by +0.01 per co-activation. The clip at 1.0 binds only if some neuron pair
co-activates >100 times; for K=20-sparse random activations over T=256 steps
the max co-activation count is ~20 (verified host-side; numpy fallback
otherwise). With clip never binding:

  sigma_t = 0.01 * X_{<t}^T X_{<t}        (X = binary acts [T,N])
  pred_t  = 0.01 * X_{<t}^T g_t,  g_t = X_{<t} x_t = G[:t, t],  G = X X^T
  <pred_t, x_t>  = 0.01 * sum_{s<t} G[s,t]^2
  |pred_t|^2     = 1e-4 * g_t^T G_{<t,<t} g_t = 1e-4 * sum_s L[s,t] (G L)[s,t]
  with L = strictly-"earlier" masked G. So the serial scan collapses into a
  few small matmuls on the token-gram matrix G [T,T].

Device pipeline (single-core program, replicated SPMD on 8 cores):
  1. dma_gather of the T projection rows (token ids baked at compile time;
     the int16 index limit is handled by splitting the vocab at 32768 and
     permuting tokens so low-vocab tokens occupy a slot prefix; the one
     mixed 128-token chunk is assembled via a parallel scratch gather and a
     partition-aligned stitch copy).
  2. Exact top-20 threshold per 1024-wide row on the DVE:
     - segmented path (validity host-verified per input): top-8 of each of
       16 64-wide segments via max8, then 3 max8 + 2 match_replace merge
       rounds over the 128 candidates; thr = 4th value of round 3.
     - fallback: 3 max8 + 2 match_replace rounds over the full row.
  3. acts = (raw >= thr) as bf16 (with per-row count via accum_out);
     PE-transpose to neuron-major XT.
  4. G = XT^T XT (PE, bf16 exact: entries are ints <= 20).
  5. L = G * mask, mask[s,t] = [time(s) < time(t)] precomputed host-side
     (handles the vocab-split token permutation).
  6. M = G @ L (PE); dot = colsum(L*L); pn2 = colsum(L*M).
  7. tension = 1 - dot / (sqrt(pn2*cnt) + 1e-6)   [identical regrouping of
     the reference's 1 - 0.01*dot / (0.01*sqrt(pn2)*sqrt(cnt) + 1e-8)].
  8. DMA out [1, T]; host un-permutes slots back to time order.
"""

import os
import numpy as np

T, N, K = 256, 1024, 20
VOCAB, HALF = 50257, 32768
NCH = N // 128   # 8 neuron chunks
TCH = T // 128   # 2 token chunks

LAST_RESULT = None  # BassKernelResults of the most recent device run


def _numpy_fallback(projection, sigma, tokens, plasticity):
    """Exact step-by-step emulation of the reference (f32). Only used if the
    fast-path preconditions fail (never, for the reference input family)."""
    proj = np.asarray(projection, np.float32)
    raw = proj[np.asarray(tokens)]
    kth = np.partition(raw, N - K, axis=1)[:, N - K]
    acts = (raw >= kth[:, None]).astype(np.float32)
    sig = np.array(sigma, np.float32, copy=True)
    out = np.zeros(T, np.float32)
    for t in range(T):
        x = acts[t]
        pred = (sig @ x).astype(np.float32)
        pn2 = np.float32(np.dot(pred, pred))
        pn = np.sqrt(pn2 if pn2 > 0 else np.float32(1.0))
        xn = np.float32(np.sqrt(np.dot(x, x)))
        overlap = np.float32(np.dot(pred, x)) / (pn * xn + np.float32(1e-8))
        out[t] = np.float32(1.0) - overlap if pn2 > 0 else np.float32(1.0)
        if plasticity:
            sig = np.clip(sig + np.float32(0.01) * np.outer(x, x), 0.0, 1.0)
    return out


def _plan_gathers(ptok, nlow):
    """Returns (gathers, stitches). Each gather: (dest, chunk, half, idxs)
    with dest in {"raw", "scr"}; all gathers write disjoint tiles and run in
    parallel. Each stitch: (chunk, part_off, rows) — a partition-aligned ACT
    copy scr[part_off:part_off+rows] -> raw_chunk[part_off:...]."""
    gathers, stitches = [], []
    for c in range(TCH):
        lc = int(np.clip(nlow - 128 * c, 0, 128))
        hc = 128 - lc
        lo = ptok[128 * c: 128 * c + lc]
        hi = ptok[128 * c + lc: 128 * (c + 1)]
        if hc == 0:
            gathers.append(("raw", c, 0, lo))
        elif lc == 0:
            gathers.append(("raw", c, 1, hi - HALF))
        else:
            # lows go to scratch partitions [0, lc); highs go straight into
            # the chunk with an lc-row junk prefix (overwritten by the
            # stitch copy, which starts at partition 0 as engines require)
            gathers.append(("scr", c, 0, lo))
            idxs = np.concatenate([np.zeros(lc, np.int64), hi - HALF])
            gathers.append(("raw", c, 1, idxs))
            stitches.append((c, 0, lc))
    return gathers, stitches


def _wrap_idxs(idxs):
    """dma_gather index layout: slot j -> row j%16, col j//16, replicated to
    128 partitions; 8 int16 columns per gather."""
    w = np.full((16, 8), -1, np.int16)
    for j, v in enumerate(idxs):
        w[j % 16, j // 16] = v
    return np.tile(w, (8, 1))


def _build(tokens_np, nseg=16):
    """Build the Bass module with token ids baked in. Returns (nc, in_map, perm)."""
    from contextlib import ExitStack
    import concourse.bacc as bacc
    import concourse.mybir as mybir
    import concourse.tile as tile
    from concourse import masks
    from concourse.tile import add_dep_helper

    dt = mybir.dt
    Alu = mybir.AluOpType
    Act = mybir.ActivationFunctionType

    tok = np.asarray(tokens_np, np.int64)
    lows = np.where(tok < HALF)[0]
    highs = np.where(tok >= HALF)[0]
    perm = np.concatenate([lows, highs])      # slot -> original position
    ptok = tok[perm]
    nlow = len(lows)
    gathers, stitches = _plan_gathers(ptok, nlow)

    gidx_np = np.concatenate([_wrap_idxs(g[3]) for g in gathers], axis=1)
    tv = perm.astype(np.float32)              # original time per slot
    # msk[m][p, t]  = 1.0 iff time(128m+p) < time(t)   (L in [s, t] layout)
    # msk2[m][p, s] = 1.0 iff time(s) < time(128m+p)   (L^T in [t, s] layout)
    msk_np = np.concatenate(
        [(tv[None, :] > tv[128 * m: 128 * (m + 1), None]).astype(np.float32)
         for m in range(TCH)], axis=1)        # [128, TCH*T]
    msk2_np = np.concatenate(
        [(tv[None, :] < tv[128 * m: 128 * (m + 1), None]).astype(np.float32)
         for m in range(TCH)], axis=1)        # [128, TCH*T]

    nc = bacc.Bacc("TRN2", target_bir_lowering=False, debug=False,
                   enable_asserts=False, num_devices=1)

    proj_d = nc.dram_tensor("proj", [VOCAB, N], dt.float32, kind="ExternalInput")
    gidx_d = nc.dram_tensor("gidx", list(gidx_np.shape), dt.int16, kind="ExternalInput")
    msk_d = nc.dram_tensor("msk", [128, TCH * T], dt.float32, kind="ExternalInput")
    msk2_d = nc.dram_tensor("msk2", [128, TCH * T], dt.float32, kind="ExternalInput")
    out_d = nc.dram_tensor("tens", [128, TCH], dt.float32, kind="ExternalOutput")

    with tile.TileContext(nc) as tc, ExitStack() as ctx:
        pool = ctx.enter_context(tc.tile_pool(name="main", bufs=1))
        ppt = ctx.enter_context(tc.tile_pool(name="ppt", bufs=4, space="PSUM"))
        pacc = ctx.enter_context(tc.tile_pool(name="pacc", bufs=1, space="PSUM"))

        raw = pool.tile([128, TCH * N], dt.float32, tag="raw")
        scr = pool.tile([128, N], dt.float32, tag="scr")
        gidx = pool.tile([128, gidx_np.shape[1]], dt.int16, tag="gidx")
        msk = pool.tile([128, TCH * T], dt.float32, tag="msk")
        msk2 = pool.tile([128, TCH * T], dt.float32, tag="msk2")
        seg_topk = nseg > 0
        cand = pool.tile([128, 8 * max(nseg, 1) * TCH], dt.float32, tag="cand")
        rawc = None if seg_topk else pool.tile([128, TCH * N], dt.float32, tag="rawc")
        m8 = pool.tile([128, 24 * TCH], dt.float32, tag="m8")
        acts = pool.tile([128, TCH * N], dt.bfloat16, tag="acts")
        ident = pool.tile([128, 128], dt.bfloat16, tag="ident")
        xt = pool.tile([128, NCH * T], dt.bfloat16, tag="xt")
        gb = pool.tile([128, TCH * T], dt.bfloat16, tag="gb")
        lt = pool.tile([128, TCH * T], dt.float32, tag="lt")
        lb = pool.tile([128, TCH * T], dt.bfloat16, tag="lb")
        dump = pool.tile([128, T], dt.float32, tag="dump")
        prod1 = pool.tile([128, TCH * T], dt.float32, tag="prod1")
        prod2 = pool.tile([128, TCH * T], dt.float32, tag="prod2")
        cnt_pm = pool.tile([128, TCH], dt.float32, tag="cnt_pm")
        dotv = pool.tile([128, TCH], dt.float32, tag="dotv")
        pn2v = pool.tile([128, TCH], dt.float32, tag="pn2v")
        q_v = pool.tile([128, TCH], dt.float32, tag="q_v")
        r_v = pool.tile([128, TCH], dt.float32, tag="r_v")
        rec_v = pool.tile([128, TCH], dt.float32, tag="rec_v")
        prod_v = pool.tile([128, TCH], dt.float32, tag="prod_v")
        tens_v = pool.tile([128, TCH], dt.float32, tag="tens_v")
        pre_v = pool.tile([128, 1], dt.float32, tag="pre_v")

        # --- constants, ACT table preloads, small input DMAs ---
        nc.sync.dma_start(gidx[:], gidx_d.ap())
        nc.sync.dma_start(msk[:], msk_d.ap())
        nc.sync.dma_start(msk2[:], msk2_d.ap())
        # preload ACT function tables off the critical path (sqrt(1)=1)
        nc.gpsimd.memset(pre_v[:], 1.0)
        nc.scalar.activation(pre_v[:], pre_v[:], Act.Copy)
        nc.scalar.activation(pre_v[:], pre_v[:], Act.Sqrt)
        masks.make_identity(nc, ident[:])

        # --- 1. gathers (all parallel; disjoint dest tiles) + stitch ---
        raw3 = raw[:].rearrange("p (c n) -> p c n", n=N)
        scr3 = scr[:].rearrange("p (c n) -> p c n", n=N)
        proj_ap = proj_d.ap()
        for g, (dest, c, half, idxs) in enumerate(gathers):
            out_ap = raw3[:, c: c + 1, :] if dest == "raw" else scr3[:, 0:1, :]
            nc.gpsimd.dma_gather(
                out_ap=out_ap,
                in_ap=proj_ap[HALF:, :] if half else proj_ap,
                idxs_ap=gidx[:, 8 * g: 8 * g + (len(idxs) + 15) // 16],
                num_idxs=len(idxs),
                num_idxs_reg=int(len(idxs)),
                elem_size=N,
            )
        for c, off, rows in stitches:
            nc.scalar.activation(
                raw[off:off + rows, c * N:(c + 1) * N],
                scr[off:off + rows, :], Act.Copy)

        # --- 2+3. per token-chunk: top-20 threshold, acts (+ row counts) ---
        prev_last = None
        for c in range(TCH):
            rc = raw[:, c * N:(c + 1) * N]
            chunk_ops = []
            if seg_topk:
                segw = N // nseg
                cd = cand[:, c * 8 * nseg:(c + 1) * 8 * nseg]
                for s in range(nseg):
                    op = nc.vector.max(
                        cd[:, s * 8:(s + 1) * 8],
                        rc[:, s * segw:(s + 1) * segw])
                    chunk_ops.append(op)
                sel = cd
            else:
                op = nc.scalar.activation(rawc[:, c * N:(c + 1) * N], rc, Act.Copy)
                sel = rawc[:, c * N:(c + 1) * N]
                rc = sel
                chunk_ops.append(op)
            m1 = m8[:, c * 24 + 0: c * 24 + 8]
            m2 = m8[:, c * 24 + 8: c * 24 + 16]
            m3 = m8[:, c * 24 + 16: c * 24 + 24]
            src = sel if seg_topk else raw[:, c * N:(c + 1) * N]
            chunk_ops.append(nc.vector.max(m1, src))
            chunk_ops.append(nc.vector.match_replace(src, m1, src, -1e30))
            chunk_ops.append(nc.vector.max(m2, src))
            chunk_ops.append(nc.vector.match_replace(src, m2, src, -1e30))
            chunk_ops.append(nc.vector.max(m3, src))
            thr = m8[:, c * 24 + 19: c * 24 + 20]   # 4th of round 3 = 20th
            last = nc.vector.tensor_scalar(
                acts[:, c * N:(c + 1) * N], rc, thr, None, Alu.is_ge,
                Alu.add, accum_out=cnt_pm[:, c: c + 1])
            chunk_ops.append(last)
            # keep the DVE chain chunk-ordered so chunk 0 finishes early and
            # its transposes/G overlap chunk 1's top-k
            if prev_last is not None:
                for op in chunk_ops:
                    add_dep_helper(op.ins, prev_last.ins, sync=False,
                                   reason="chunk-order DVE chain")
            prev_last = last

        # --- 3b. PE transpose acts -> XT [neuron, token] (bf16) ---
        # blocks grouped by token-half r so all r=0 work (transpose, copy,
        # and the G half-matmuls below) overlaps chunk 1's top-k; four
        # 128x128 transposes pack into one PSUM tile so one wide copy
        # evacuates them. xt free layout: index = r*N + cn*128.
        for r in range(TCH):
            for g in range(NCH // 4):
                pt = ppt.tile([128, 512], dt.bfloat16, tag="pt")
                for j in range(4):
                    cn = g * 4 + j
                    nc.tensor.transpose(
                        pt[:, j * 128:(j + 1) * 128],
                        acts[:, r * N + cn * 128: r * N + (cn + 1) * 128],
                        ident[:],
                    )
                dst = xt[:, r * N + g * 512: r * N + (g + 1) * 512]
                if r == 0 or g % 2 == 0:
                    # ACT: the DVE must not be interrupted mid-top-k (r=0)
                    nc.scalar.activation(dst, pt[:], Act.Copy)
                else:
                    nc.vector.tensor_copy(dst, pt[:])

        # --- 4. G = X X^T  [T, T] f32 psum, via bf16 matmuls (exact),
        #        split by token-half r so the r=0 half runs early ---
        gps = []
        for m in range(TCH):
            gp = pacc.tile([128, T], dt.float32, tag=f"g{m}")
            gps.append(gp)
        for r in range(TCH):
            for m in range(TCH):
                for cn in range(NCH):
                    nc.tensor.matmul(
                        gps[m][:, r * 128:(r + 1) * 128],
                        xt[:, m * N + cn * 128: m * N + (cn + 1) * 128],
                        xt[:, r * N + cn * 128: r * N + (cn + 1) * 128],
                        start=(cn == 0), stop=(cn == NCH - 1),
                    )

        # --- 5+6. masked prefix matrices straight from PSUM, M^T = L^T G,
        #        and the dot/pn2 row reductions — all split by token-half so
        #        every piece gated only on r=0 data runs during chunk 1's
        #        top-k. Emission order == dependency order (r ascending).
        #        lb = bf16(G * msk)   (L, [s, t] layout — lhsT for M^T)
        #        lt = f32 (G * msk2)  (L^T, [t, s] layout — for row TTRs)
        #        gb = bf16(G)         (rhs for M^T) ---
        mts = []
        for m in range(TCH):
            mt = pacc.tile([128, T], dt.float32, tag=f"mt{m}")
            mts.append(mt)

        # gb halves by r (ACT — free during chunk 1's top-k); lb/lt as full
        # DVE ops (DVE is the serial resource; splitting only adds overhead)
        for r in range(TCH):
            for m in range(TCH):
                sl = slice(m * T + r * 128, m * T + (r + 1) * 128)
                nc.scalar.activation(gb[:, sl],
                                     gps[m][:, r * 128:(r + 1) * 128], Act.Copy)
        # lb first: it unblocks the M^T matmuls on the PE
        for m in range(TCH):
            nc.vector.tensor_mul(lb[:, m * T:(m + 1) * T], gps[m][:],
                                 msk[:, m * T:(m + 1) * T])
        # NOTE: tensor_tensor_reduce is rejected by this runtime (device
        # NRT_EXEC_UNIT_UNRECOVERABLE) — reductions use an exact DVE product
        # followed by an ACT Copy with accum_out (HW-verified) instead.
        for m in range(TCH):
            ltm = lt[:, m * T:(m + 1) * T]
            nc.vector.tensor_mul(ltm, gps[m][:], msk2[:, m * T:(m + 1) * T])
            # dot[t] = sum_s L^T[t,s]^2 — off the critical DVE sequence
            # (GPSIMD product; dot only gates the final subtract)
            nc.gpsimd.tensor_mul(prod1[:, m * T:(m + 1) * T], ltm, ltm)
            nc.scalar.activation(dump[:], prod1[:, m * T:(m + 1) * T],
                                 Act.Copy, accum_out=dotv[:, m: m + 1])
        for m in range(TCH):
            for b in range(TCH):
                nc.tensor.matmul(
                    mts[m][:],
                    lb[:, b * T + m * 128: b * T + (m + 1) * 128],
                    gb[:, b * T:(b + 1) * T],
                    start=(b == 0), stop=(b == TCH - 1),
                )
            nc.vector.tensor_mul(prod2[:, m * T:(m + 1) * T],
                                 lt[:, m * T:(m + 1) * T], mts[m][:])
            # scale = cnt folds q = pn2*cnt into the accumulate (exact: all
            # terms are integers < 2^24), so sqrt follows directly on ACT
            nc.scalar.activation(dump[:], prod2[:, m * T:(m + 1) * T],
                                 Act.Copy, scale=cnt_pm[:, m: m + 1],
                                 accum_out=q_v[:, m: m + 1])

        # --- 7. final per-token math on [128, TCH] (token-major):
        #     tension = 1 - dot/denom = (denom - dot)/denom,
        #     denom = sqrt(pn2*cnt) + 1e-6; q = pn2*cnt from the accum above ---
        nc.scalar.activation(r_v[:], q_v[:], Act.Sqrt)
        nc.vector.tensor_scalar_add(r_v[:], r_v[:], 1e-6)
        nc.vector.tensor_tensor(prod_v[:], r_v[:], dotv[:], Alu.subtract)
        nc.vector.reciprocal(rec_v[:], r_v[:])
        nc.vector.tensor_mul(tens_v[:], prod_v[:], rec_v[:])

        # --- 8. output: plain [128, TCH] DMA; host maps (p, c) -> t = 128c+p ---
        nc.sync.dma_start(out_d.ap(), tens_v[:])

    nc.compile()

    in_map = {
        "proj": None,  # filled by caller (f32 [VOCAB, N])
        "gidx": gidx_np,
        "msk": msk_np,
        "msk2": msk2_np,
    }
    return nc, in_map, perm


def _check_input(projection, sigma, tokens):
    """Host-side guards. Returns (fast_ok, nseg):
    fast_ok — the algebraic rewrite is exact (sigma==0, clip never binds);
    nseg    — widest valid segmentation for the segmented top-k (a
    segmentation is valid when taking the top-8 of every segment still
    captures all of each row's top-20 values), or 0 for the full-row path."""
    if np.any(np.asarray(sigma)):
        return False, 0
    proj = np.asarray(projection, np.float32)
    raw = proj[np.asarray(tokens)]
    kth = np.partition(raw, N - K, axis=1)[:, N - K]
    acts = (raw >= kth[:, None]).astype(np.float32)
    coact = acts.T @ acts
    fast_ok = float(coact.max()) <= 100.0
    nseg = 0
    for cand_nseg in (8, 16):
        segs = raw.reshape(T, cand_nseg, N // cand_nseg)
        cand = -np.sort(-segs, axis=2)[:, :, :8].reshape(T, cand_nseg * 8)
        thr_dev = -np.sort(-cand, axis=1)[:, K - 1]
        if bool(np.all(thr_dev == kth)):
            nseg = cand_nseg
            break
    return fast_ok, nseg


def kernel(projection, sigma, tokens, plasticity):
    global LAST_RESULT
    projection = np.ascontiguousarray(np.asarray(projection, np.float32))
    sigma = np.asarray(sigma, np.float32)
    tokens = np.asarray(tokens).astype(np.int64)
    plast = int(np.asarray(plasticity).reshape(-1)[0]) if np.ndim(plasticity) else int(plasticity)

    if not plast:
        # sigma never updates; with sigma == 0, pred == 0 -> tension == 1.
        if not np.any(sigma):
            return np.ones(T, np.float32)
        return _numpy_fallback(projection, sigma, tokens, plast)
    fast_ok, nseg = _check_input(projection, sigma, tokens)
    if not fast_ok:
        return _numpy_fallback(projection, sigma, tokens, plast)

    from concourse.bass_utils import run_bass_kernel_spmd

    nc, in_map, perm = _build(tokens, nseg=nseg)
    in_map["proj"] = projection
    n_cores = int(os.environ.get("BDH_CORES", "8"))
    try:
        res = run_bass_kernel_spmd(
            nc,
            [dict(in_map) for _ in range(n_cores)],
            core_ids=list(range(n_cores)),
        )
    except ModuleNotFoundError:
        # BASS_TRACE was requested but this axon build has no NTFF hook.
        os.environ["BASS_NEVER_TRACE"] = "1"
        res = run_bass_kernel_spmd(
            nc,
            [dict(in_map) for _ in range(n_cores)],
            core_ids=list(range(n_cores)),
        )
    LAST_RESULT = res
    # device layout [p, c] -> slot t = 128c + p; then slot -> original time
    tens_slots = res.results[0]["tens"].reshape(128, TCH).T.reshape(T)
    out = np.empty(T, np.float32)
    out[perm] = tens_slots.astype(np.float32)
    return out


# revision 44
# speedup vs baseline: 1.5910x; 1.0169x over previous
"""Trainium2 Bass kernel for nn_BDHModel (scatter_memory).

Computes, for T tokens:
  raw  = projection[tokens]                  # [T, N] gather
  thr  = 20th largest per row; acts = raw >= thr   (binary, K=20 active)
  scan: pred = sigma @ x; tension_t = 1 - <pred,x>/(|pred||x|+1e-8);
        sigma += 0.01 * outer(x,x), clipped to [0,1]

Key algebraic identity used on device: sigma starts at 0 and each entry grows
by +0.01 per co-activation. The clip at 1.0 binds only if some neuron pair
co-activates >100 times; for K=20-sparse random activations over T=256 steps
the max co-activation count is ~20 (verified host-side; numpy fallback
otherwise). With clip never binding:

  sigma_t = 0.01 * X_{<t}^T X_{<t}        (X = binary acts [T,N])
  pred_t  = 0.01 * X_{<t}^T g_t,  g_t = X_{<t} x_t = G[:t, t],  G = X X^T
  <pred_t, x_t>  = 0.01 * sum_{s<t} G[s,t]^2
  |pred_t|^2     = 1e-4 * g_t^T G_{<t,<t} g_t = 1e-4 * sum_s L[s,t] (G L)[s,t]
  with L = strictly-"earlier" masked G. So the serial scan collapses into a
  few small matmuls on the token-gram matrix G [T,T].

Device pipeline (single-core program, replicated SPMD on 8 cores):
  1. dma_gather of the T projection rows (token ids baked at compile time;
     the int16 index limit is handled by splitting the vocab at 32768 and
     permuting tokens so low-vocab tokens occupy a slot prefix; the one
     mixed 128-token chunk is assembled via a parallel scratch gather and a
     partition-aligned stitch copy).
  2. Exact top-20 threshold per 1024-wide row on the DVE:
     - segmented path (validity host-verified per input): top-8 of each of
       16 64-wide segments via max8, then 3 max8 + 2 match_replace merge
       rounds over the 128 candidates; thr = 4th value of round 3.
     - fallback: 3 max8 + 2 match_replace rounds over the full row.
  3. acts = (raw >= thr) as bf16 (with per-row count via accum_out);
     PE-transpose to neuron-major XT.
  4. G = XT^T XT (PE, bf16 exact: entries are ints <= 20).
  5. L = G * mask, mask[s,t] = [time(s) < time(t)] precomputed host-side
     (handles the vocab-split token permutation).
  6. M = G @ L (PE); dot = colsum(L*L); pn2 = colsum(L*M).
  7. tension = 1 - dot / (sqrt(pn2*cnt) + 1e-6)   [identical regrouping of
     the reference's 1 - 0.01*dot / (0.01*sqrt(pn2)*sqrt(cnt) + 1e-8)].
  8. DMA out [1, T]; host un-permutes slots back to time order.
"""

import os
import numpy as np

T, N, K = 256, 1024, 20
VOCAB, HALF = 50257, 32768
NCH = N // 128   # 8 neuron chunks
TCH = T // 128   # 2 token chunks

LAST_RESULT = None  # BassKernelResults of the most recent device run


def _numpy_fallback(projection, sigma, tokens, plasticity):
    """Exact step-by-step emulation of the reference (f32). Only used if the
    fast-path preconditions fail (never, for the reference input family)."""
    proj = np.asarray(projection, np.float32)
    raw = proj[np.asarray(tokens)]
    kth = np.partition(raw, N - K, axis=1)[:, N - K]
    acts = (raw >= kth[:, None]).astype(np.float32)
    sig = np.array(sigma, np.float32, copy=True)
    out = np.zeros(T, np.float32)
    for t in range(T):
        x = acts[t]
        pred = (sig @ x).astype(np.float32)
        pn2 = np.float32(np.dot(pred, pred))
        pn = np.sqrt(pn2 if pn2 > 0 else np.float32(1.0))
        xn = np.float32(np.sqrt(np.dot(x, x)))
        overlap = np.float32(np.dot(pred, x)) / (pn * xn + np.float32(1e-8))
        out[t] = np.float32(1.0) - overlap if pn2 > 0 else np.float32(1.0)
        if plasticity:
            sig = np.clip(sig + np.float32(0.01) * np.outer(x, x), 0.0, 1.0)
    return out


def _plan_gathers(ptok, nlow):
    """Returns (gathers, stitches). Each gather: (dest, chunk, half, idxs)
    with dest in {"raw", "scr"}; all gathers write disjoint tiles and run in
    parallel. Each stitch: (chunk, part_off, rows) — a partition-aligned ACT
    copy scr[part_off:part_off+rows] -> raw_chunk[part_off:...]."""
    gathers, stitches = [], []
    for c in range(TCH):
        lc = int(np.clip(nlow - 128 * c, 0, 128))
        hc = 128 - lc
        lo = ptok[128 * c: 128 * c + lc]
        hi = ptok[128 * c + lc: 128 * (c + 1)]
        if hc == 0:
            gathers.append(("raw", c, 0, lo))
        elif lc == 0:
            gathers.append(("raw", c, 1, hi - HALF))
        else:
            # lows go to scratch partitions [0, lc); highs go straight into
            # the chunk with an lc-row junk prefix (overwritten by the
            # stitch copy, which starts at partition 0 as engines require)
            gathers.append(("scr", c, 0, lo))
            idxs = np.concatenate([np.zeros(lc, np.int64), hi - HALF])
            gathers.append(("raw", c, 1, idxs))
            stitches.append((c, 0, lc))
    return gathers, stitches


def _wrap_idxs(idxs):
    """dma_gather index layout: slot j -> row j%16, col j//16, replicated to
    128 partitions; 8 int16 columns per gather."""
    w = np.full((16, 8), -1, np.int16)
    for j, v in enumerate(idxs):
        w[j % 16, j // 16] = v
    return np.tile(w, (8, 1))


def _build(tokens_np, nseg=16):
    """Build the Bass module with token ids baked in. Returns (nc, in_map, perm)."""
    from contextlib import ExitStack
    import concourse.bacc as bacc
    import concourse.mybir as mybir
    import concourse.tile as tile
    from concourse import masks
    from concourse.tile import add_dep_helper

    dt = mybir.dt
    Alu = mybir.AluOpType
    Act = mybir.ActivationFunctionType

    tok = np.asarray(tokens_np, np.int64)
    lows = np.where(tok < HALF)[0]
    highs = np.where(tok >= HALF)[0]
    perm = np.concatenate([lows, highs])      # slot -> original position
    ptok = tok[perm]
    nlow = len(lows)
    gathers, stitches = _plan_gathers(ptok, nlow)

    gidx_np = np.concatenate([_wrap_idxs(g[3]) for g in gathers], axis=1)
    tv = perm.astype(np.float32)              # original time per slot
    # msk[m][p, t]  = 1.0 iff time(128m+p) < time(t)   (L in [s, t] layout)
    # msk2[m][p, s] = 1.0 iff time(s) < time(128m+p)   (L^T in [t, s] layout)
    msk_np = np.concatenate(
        [(tv[None, :] > tv[128 * m: 128 * (m + 1), None]).astype(np.float32)
         for m in range(TCH)], axis=1)        # [128, TCH*T]
    msk2_np = np.concatenate(
        [(tv[None, :] < tv[128 * m: 128 * (m + 1), None]).astype(np.float32)
         for m in range(TCH)], axis=1)        # [128, TCH*T]

    nc = bacc.Bacc("TRN2", target_bir_lowering=False, debug=False,
                   enable_asserts=False, num_devices=1)

    proj_d = nc.dram_tensor("proj", [VOCAB, N], dt.float32, kind="ExternalInput")
    gidx_d = nc.dram_tensor("gidx", list(gidx_np.shape), dt.int16, kind="ExternalInput")
    msk_d = nc.dram_tensor("msk", [128, TCH * T], dt.float32, kind="ExternalInput")
    msk2_d = nc.dram_tensor("msk2", [128, TCH * T], dt.float32, kind="ExternalInput")
    out_d = nc.dram_tensor("tens", [128, TCH], dt.float32, kind="ExternalOutput")

    with tile.TileContext(nc) as tc, ExitStack() as ctx:
        pool = ctx.enter_context(tc.tile_pool(name="main", bufs=1))
        ppt = ctx.enter_context(tc.tile_pool(name="ppt", bufs=4, space="PSUM"))
        pacc = ctx.enter_context(tc.tile_pool(name="pacc", bufs=1, space="PSUM"))

        raw = pool.tile([128, TCH * N], dt.float32, tag="raw")
        scr = pool.tile([128, N], dt.float32, tag="scr")
        gidx = pool.tile([128, gidx_np.shape[1]], dt.int16, tag="gidx")
        msk = pool.tile([128, TCH * T], dt.float32, tag="msk")
        msk2 = pool.tile([128, TCH * T], dt.float32, tag="msk2")
        seg_topk = nseg > 0
        cand = pool.tile([128, 8 * max(nseg, 1) * TCH], dt.float32, tag="cand")
        rawc = None if seg_topk else pool.tile([128, TCH * N], dt.float32, tag="rawc")
        m8 = pool.tile([128, 24 * TCH], dt.float32, tag="m8")
        acts = pool.tile([128, TCH * N], dt.bfloat16, tag="acts")
        ident = pool.tile([128, 128], dt.bfloat16, tag="ident")
        xt = pool.tile([128, NCH * T], dt.bfloat16, tag="xt")
        gb = pool.tile([128, TCH * T], dt.bfloat16, tag="gb")
        lt = pool.tile([128, TCH * T], dt.float32, tag="lt")
        lb = pool.tile([128, TCH * T], dt.bfloat16, tag="lb")
        dump = pool.tile([128, T], dt.float32, tag="dump")
        prod1 = pool.tile([128, TCH * T], dt.float32, tag="prod1")
        prod2 = pool.tile([128, TCH * T], dt.float32, tag="prod2")
        cnt_pm = pool.tile([128, TCH], dt.float32, tag="cnt_pm")
        dotv = pool.tile([128, TCH], dt.float32, tag="dotv")
        pn2v = pool.tile([128, TCH], dt.float32, tag="pn2v")
        q_v = pool.tile([128, TCH], dt.float32, tag="q_v")
        r_v = pool.tile([128, TCH], dt.float32, tag="r_v")
        rec_v = pool.tile([128, TCH], dt.float32, tag="rec_v")
        prod_v = pool.tile([128, TCH], dt.float32, tag="prod_v")
        tens_v = pool.tile([128, TCH], dt.float32, tag="tens_v")
        pre_v = pool.tile([128, 1], dt.float32, tag="pre_v")

        # --- constants, ACT table preloads, small input DMAs ---
        nc.sync.dma_start(gidx[:], gidx_d.ap())
        nc.sync.dma_start(msk[:], msk_d.ap())
        nc.sync.dma_start(msk2[:], msk2_d.ap())
        # preload ACT function tables off the critical path (sqrt(1)=1)
        nc.gpsimd.memset(pre_v[:], 1.0)
        nc.scalar.activation(pre_v[:], pre_v[:], Act.Copy)
        nc.scalar.activation(pre_v[:], pre_v[:], Act.Sqrt)
        masks.make_identity(nc, ident[:])

        # --- 1. gathers (all parallel; disjoint dest tiles) + stitch ---
        raw3 = raw[:].rearrange("p (c n) -> p c n", n=N)
        scr3 = scr[:].rearrange("p (c n) -> p c n", n=N)
        proj_ap = proj_d.ap()
        for g, (dest, c, half, idxs) in enumerate(gathers):
            out_ap = raw3[:, c: c + 1, :] if dest == "raw" else scr3[:, 0:1, :]
            nc.gpsimd.dma_gather(
                out_ap=out_ap,
                in_ap=proj_ap[HALF:, :] if half else proj_ap,
                idxs_ap=gidx[:, 8 * g: 8 * g + (len(idxs) + 15) // 16],
                num_idxs=len(idxs),
                num_idxs_reg=int(len(idxs)),
                elem_size=N,
            )
        for c, off, rows in stitches:
            nc.scalar.activation(
                raw[off:off + rows, c * N:(c + 1) * N],
                scr[off:off + rows, :], Act.Copy)

        # --- 2+3. per token-chunk: top-20 threshold, acts (+ row counts) ---
        prev_last = None
        for c in range(TCH):
            rc = raw[:, c * N:(c + 1) * N]
            chunk_ops = []
            if seg_topk:
                segw = N // nseg
                cd = cand[:, c * 8 * nseg:(c + 1) * 8 * nseg]
                for s in range(nseg):
                    op = nc.vector.max(
                        cd[:, s * 8:(s + 1) * 8],
                        rc[:, s * segw:(s + 1) * segw])
                    chunk_ops.append(op)
                sel = cd
            else:
                op = nc.scalar.activation(rawc[:, c * N:(c + 1) * N], rc, Act.Copy)
                sel = rawc[:, c * N:(c + 1) * N]
                rc = sel
                chunk_ops.append(op)
            m1 = m8[:, c * 24 + 0: c * 24 + 8]
            m2 = m8[:, c * 24 + 8: c * 24 + 16]
            m3 = m8[:, c * 24 + 16: c * 24 + 24]
            src = sel if seg_topk else raw[:, c * N:(c + 1) * N]
            chunk_ops.append(nc.vector.max(m1, src))
            chunk_ops.append(nc.vector.match_replace(src, m1, src, -1e30))
            chunk_ops.append(nc.vector.max(m2, src))
            chunk_ops.append(nc.vector.match_replace(src, m2, src, -1e30))
            chunk_ops.append(nc.vector.max(m3, src))
            thr = m8[:, c * 24 + 19: c * 24 + 20]   # 4th of round 3 = 20th
            last = nc.vector.tensor_scalar(
                acts[:, c * N:(c + 1) * N], rc, thr, None, Alu.is_ge,
                Alu.add, accum_out=cnt_pm[:, c: c + 1])
            chunk_ops.append(last)
            # keep the DVE chain chunk-ordered so chunk 0 finishes early and
            # its transposes/G overlap chunk 1's top-k
            if prev_last is not None:
                for op in chunk_ops:
                    add_dep_helper(op.ins, prev_last.ins, sync=False,
                                   reason="chunk-order DVE chain")
            prev_last = last

        # --- 3b. PE transpose acts -> XT [neuron, token] (bf16) ---
        # blocks grouped by token-half r so all r=0 work (transpose, copy,
        # and the G half-matmuls below) overlaps chunk 1's top-k; four
        # 128x128 transposes pack into one PSUM tile so one wide copy
        # evacuates them. xt free layout: index = r*N + cn*128.
        for r in range(TCH):
            for g in range(NCH // 4):
                pt = ppt.tile([128, 512], dt.bfloat16, tag="pt")
                for j in range(4):
                    cn = g * 4 + j
                    nc.tensor.transpose(
                        pt[:, j * 128:(j + 1) * 128],
                        acts[:, r * N + cn * 128: r * N + (cn + 1) * 128],
                        ident[:],
                    )
                dst = xt[:, r * N + g * 512: r * N + (g + 1) * 512]
                if r == 0 or g % 2 == 0:
                    # ACT: the DVE must not be interrupted mid-top-k (r=0)
                    nc.scalar.activation(dst, pt[:], Act.Copy)
                else:
                    nc.vector.tensor_copy(dst, pt[:])

        # --- 4. G = X X^T  [T, T] f32 psum, via bf16 matmuls (exact),
        #        split by token-half r so the r=0 half runs early ---
        gps = []
        for m in range(TCH):
            gp = pacc.tile([128, T], dt.float32, tag=f"g{m}")
            gps.append(gp)
        # m-outer: gps[0] completes first so the DVE's masked muls (below)
        # start while gps[1]'s groups are still on the PE
        for m in range(TCH):
            for r in range(TCH):
                for cn in range(NCH):
                    nc.tensor.matmul(
                        gps[m][:, r * 128:(r + 1) * 128],
                        xt[:, m * N + cn * 128: m * N + (cn + 1) * 128],
                        xt[:, r * N + cn * 128: r * N + (cn + 1) * 128],
                        start=(cn == 0), stop=(cn == NCH - 1),
                    )

        # --- 5+6. masked prefix matrices straight from PSUM, M^T = L^T G,
        #        and the dot/pn2 row reductions — all split by token-half so
        #        every piece gated only on r=0 data runs during chunk 1's
        #        top-k. Emission order == dependency order (r ascending).
        #        lb = bf16(G * msk)   (L, [s, t] layout — lhsT for M^T)
        #        lt = f32 (G * msk2)  (L^T, [t, s] layout — for row TTRs)
        #        gb = bf16(G)         (rhs for M^T) ---
        mts = []
        for m in range(TCH):
            mt = pacc.tile([128, T], dt.float32, tag=f"mt{m}")
            mts.append(mt)

        # gb halves by r (ACT — free during chunk 1's top-k); lb/lt as full
        # DVE ops (DVE is the serial resource; splitting only adds overhead)
        for r in range(TCH):
            for m in range(TCH):
                sl = slice(m * T + r * 128, m * T + (r + 1) * 128)
                nc.scalar.activation(gb[:, sl],
                                     gps[m][:, r * 128:(r + 1) * 128], Act.Copy)
        # NOTE: tensor_tensor_reduce is rejected by this runtime (device
        # NRT_EXEC_UNIT_UNRECOVERABLE) — reductions use an exact DVE product
        # followed by an ACT Copy with accum_out (HW-verified) instead.
        # Per-block interleave: all m=0 work is emitted before anything
        # gated on gps[1], so the DVE isn't head-of-line blocked.
        for m in range(TCH):
            ltm = lt[:, m * T:(m + 1) * T]
            nc.vector.tensor_mul(lb[:, m * T:(m + 1) * T], gps[m][:],
                                 msk[:, m * T:(m + 1) * T])
            nc.vector.tensor_mul(ltm, gps[m][:], msk2[:, m * T:(m + 1) * T])
            # dot[t] = sum_s L^T[t,s]^2 — off the critical DVE sequence
            # (GPSIMD product; dot only gates the final subtract)
            nc.gpsimd.tensor_mul(prod1[:, m * T:(m + 1) * T], ltm, ltm)
            nc.scalar.activation(dump[:], prod1[:, m * T:(m + 1) * T],
                                 Act.Copy, accum_out=dotv[:, m: m + 1])
        for m in range(TCH):
            for b in range(TCH):
                nc.tensor.matmul(
                    mts[m][:],
                    lb[:, b * T + m * 128: b * T + (m + 1) * 128],
                    gb[:, b * T:(b + 1) * T],
                    start=(b == 0), stop=(b == TCH - 1),
                )
            nc.vector.tensor_mul(prod2[:, m * T:(m + 1) * T],
                                 lt[:, m * T:(m + 1) * T], mts[m][:])
            # scale = cnt folds q = pn2*cnt into the accumulate (exact: all
            # terms are integers < 2^24), so sqrt follows directly on ACT
            nc.scalar.activation(dump[:], prod2[:, m * T:(m + 1) * T],
                                 Act.Copy, scale=cnt_pm[:, m: m + 1],
                                 accum_out=q_v[:, m: m + 1])

        # --- 7. final per-token math on [128, TCH] (token-major):
        #     tension = 1 - dot/denom = (denom - dot)/denom,
        #     denom = sqrt(pn2*cnt) + 1e-6; q = pn2*cnt from the accum above ---
        nc.scalar.activation(r_v[:], q_v[:], Act.Sqrt)
        nc.vector.tensor_scalar_add(r_v[:], r_v[:], 1e-6)
        nc.vector.tensor_tensor(prod_v[:], r_v[:], dotv[:], Alu.subtract)
        nc.vector.reciprocal(rec_v[:], r_v[:])
        nc.vector.tensor_mul(tens_v[:], prod_v[:], rec_v[:])

        # --- 8. output: plain [128, TCH] DMA; host maps (p, c) -> t = 128c+p ---
        nc.sync.dma_start(out_d.ap(), tens_v[:])

    nc.compile()

    in_map = {
        "proj": None,  # filled by caller (f32 [VOCAB, N])
        "gidx": gidx_np,
        "msk": msk_np,
        "msk2": msk2_np,
    }
    return nc, in_map, perm


def _check_input(projection, sigma, tokens):
    """Host-side guards. Returns (fast_ok, nseg):
    fast_ok — the algebraic rewrite is exact (sigma==0, clip never binds);
    nseg    — widest valid segmentation for the segmented top-k (a
    segmentation is valid when taking the top-8 of every segment still
    captures all of each row's top-20 values), or 0 for the full-row path."""
    if np.any(np.asarray(sigma)):
        return False, 0
    proj = np.asarray(projection, np.float32)
    raw = proj[np.asarray(tokens)]
    kth = np.partition(raw, N - K, axis=1)[:, N - K]
    acts = (raw >= kth[:, None]).astype(np.float32)
    coact = acts.T @ acts
    fast_ok = float(coact.max()) <= 100.0
    nseg = 0
    for cand_nseg in (8, 16):
        segs = raw.reshape(T, cand_nseg, N // cand_nseg)
        cand = -np.sort(-segs, axis=2)[:, :, :8].reshape(T, cand_nseg * 8)
        thr_dev = -np.sort(-cand, axis=1)[:, K - 1]
        if bool(np.all(thr_dev == kth)):
            nseg = cand_nseg
            break
    return fast_ok, nseg


def kernel(projection, sigma, tokens, plasticity):
    global LAST_RESULT
    projection = np.ascontiguousarray(np.asarray(projection, np.float32))
    sigma = np.asarray(sigma, np.float32)
    tokens = np.asarray(tokens).astype(np.int64)
    plast = int(np.asarray(plasticity).reshape(-1)[0]) if np.ndim(plasticity) else int(plasticity)

    if not plast:
        # sigma never updates; with sigma == 0, pred == 0 -> tension == 1.
        if not np.any(sigma):
            return np.ones(T, np.float32)
        return _numpy_fallback(projection, sigma, tokens, plast)
    fast_ok, nseg = _check_input(projection, sigma, tokens)
    if not fast_ok:
        return _numpy_fallback(projection, sigma, tokens, plast)

    from concourse.bass_utils import run_bass_kernel_spmd

    nc, in_map, perm = _build(tokens, nseg=nseg)
    in_map["proj"] = projection
    n_cores = int(os.environ.get("BDH_CORES", "8"))
    try:
        res = run_bass_kernel_spmd(
            nc,
            [dict(in_map) for _ in range(n_cores)],
            core_ids=list(range(n_cores)),
        )
    except ModuleNotFoundError:
        # BASS_TRACE was requested but this axon build has no NTFF hook.
        os.environ["BASS_NEVER_TRACE"] = "1"
        res = run_bass_kernel_spmd(
            nc,
            [dict(in_map) for _ in range(n_cores)],
            core_ids=list(range(n_cores)),
        )
    LAST_RESULT = res
    # device layout [p, c] -> slot t = 128c + p; then slot -> original time
    tens_slots = res.results[0]["tens"].reshape(128, TCH).T.reshape(T)
    out = np.empty(T, np.float32)
    out[perm] = tens_slots.astype(np.float32)
    return out


# revision 45
# speedup vs baseline: 1.6364x; 1.0286x over previous
"""Trainium2 Bass kernel for nn_BDHModel (scatter_memory).

Computes, for T tokens:
  raw  = projection[tokens]                  # [T, N] gather
  thr  = 20th largest per row; acts = raw >= thr   (binary, K=20 active)
  scan: pred = sigma @ x; tension_t = 1 - <pred,x>/(|pred||x|+1e-8);
        sigma += 0.01 * outer(x,x), clipped to [0,1]

Key algebraic identity used on device: sigma starts at 0 and each entry grows
by +0.01 per co-activation. The clip at 1.0 binds only if some neuron pair
co-activates >100 times; for K=20-sparse random activations over T=256 steps
the max co-activation count is ~20 (verified host-side; numpy fallback
otherwise). With clip never binding:

  sigma_t = 0.01 * X_{<t}^T X_{<t}        (X = binary acts [T,N])
  pred_t  = 0.01 * X_{<t}^T g_t,  g_t = X_{<t} x_t = G[:t, t],  G = X X^T
  <pred_t, x_t>  = 0.01 * sum_{s<t} G[s,t]^2
  |pred_t|^2     = 1e-4 * g_t^T G_{<t,<t} g_t = 1e-4 * sum_s L[s,t] (G L)[s,t]
  with L = strictly-"earlier" masked G. So the serial scan collapses into a
  few small matmuls on the token-gram matrix G [T,T].

Device pipeline (single-core program, replicated SPMD on 8 cores):
  1. dma_gather of the T projection rows (token ids baked at compile time;
     the int16 index limit is handled by splitting the vocab at 32768 and
     permuting tokens so low-vocab tokens occupy a slot prefix; the one
     mixed 128-token chunk is assembled via a parallel scratch gather and a
     partition-aligned stitch copy).
  2. Exact top-20 threshold per 1024-wide row on the DVE:
     - segmented path (validity host-verified per input): top-8 of each of
       16 64-wide segments via max8, then 3 max8 + 2 match_replace merge
       rounds over the 128 candidates; thr = 4th value of round 3.
     - fallback: 3 max8 + 2 match_replace rounds over the full row.
  3. acts = (raw >= thr) as bf16 (with per-row count via accum_out);
     PE-transpose to neuron-major XT.
  4. G = XT^T XT (PE, bf16 exact: entries are ints <= 20).
  5. L = G * mask, mask[s,t] = [time(s) < time(t)] precomputed host-side
     (handles the vocab-split token permutation).
  6. M = G @ L (PE); dot = colsum(L*L); pn2 = colsum(L*M).
  7. tension = 1 - dot / (sqrt(pn2*cnt) + 1e-6)   [identical regrouping of
     the reference's 1 - 0.01*dot / (0.01*sqrt(pn2)*sqrt(cnt) + 1e-8)].
  8. DMA out [1, T]; host un-permutes slots back to time order.
"""

import os
import numpy as np

T, N, K = 256, 1024, 20
VOCAB, HALF = 50257, 32768
NCH = N // 128   # 8 neuron chunks
TCH = T // 128   # 2 token chunks

LAST_RESULT = None  # BassKernelResults of the most recent device run


def _numpy_fallback(projection, sigma, tokens, plasticity):
    """Exact step-by-step emulation of the reference (f32). Only used if the
    fast-path preconditions fail (never, for the reference input family)."""
    proj = np.asarray(projection, np.float32)
    raw = proj[np.asarray(tokens)]
    kth = np.partition(raw, N - K, axis=1)[:, N - K]
    acts = (raw >= kth[:, None]).astype(np.float32)
    sig = np.array(sigma, np.float32, copy=True)
    out = np.zeros(T, np.float32)
    for t in range(T):
        x = acts[t]
        pred = (sig @ x).astype(np.float32)
        pn2 = np.float32(np.dot(pred, pred))
        pn = np.sqrt(pn2 if pn2 > 0 else np.float32(1.0))
        xn = np.float32(np.sqrt(np.dot(x, x)))
        overlap = np.float32(np.dot(pred, x)) / (pn * xn + np.float32(1e-8))
        out[t] = np.float32(1.0) - overlap if pn2 > 0 else np.float32(1.0)
        if plasticity:
            sig = np.clip(sig + np.float32(0.01) * np.outer(x, x), 0.0, 1.0)
    return out


def _plan_gathers(ptok, nlow):
    """Returns (gathers, stitches). Each gather: (dest, chunk, half, idxs)
    with dest in {"raw", "scr"}; all gathers write disjoint tiles and run in
    parallel. Each stitch: (chunk, part_off, rows) — a partition-aligned ACT
    copy scr[part_off:part_off+rows] -> raw_chunk[part_off:...]."""
    gathers, stitches = [], []
    for c in range(TCH):
        lc = int(np.clip(nlow - 128 * c, 0, 128))
        hc = 128 - lc
        lo = ptok[128 * c: 128 * c + lc]
        hi = ptok[128 * c + lc: 128 * (c + 1)]
        if hc == 0:
            gathers.append(("raw", c, 0, lo))
        elif lc == 0:
            gathers.append(("raw", c, 1, hi - HALF))
        else:
            # lows go to scratch partitions [0, lc); highs go straight into
            # the chunk with an lc-row junk prefix (overwritten by the
            # stitch copy, which starts at partition 0 as engines require)
            gathers.append(("scr", c, 0, lo))
            idxs = np.concatenate([np.zeros(lc, np.int64), hi - HALF])
            gathers.append(("raw", c, 1, idxs))
            stitches.append((c, 0, lc))
    return gathers, stitches


def _wrap_idxs(idxs):
    """dma_gather index layout: slot j -> row j%16, col j//16, replicated to
    128 partitions; 8 int16 columns per gather."""
    w = np.full((16, 8), -1, np.int16)
    for j, v in enumerate(idxs):
        w[j % 16, j // 16] = v
    return np.tile(w, (8, 1))


def _build(tokens_np, nseg=16):
    """Build the Bass module with token ids baked in. Returns (nc, in_map, perm)."""
    from contextlib import ExitStack
    import concourse.bacc as bacc
    import concourse.mybir as mybir
    import concourse.tile as tile
    from concourse import masks
    from concourse.tile import add_dep_helper

    dt = mybir.dt
    Alu = mybir.AluOpType
    Act = mybir.ActivationFunctionType

    tok = np.asarray(tokens_np, np.int64)
    lows = np.where(tok < HALF)[0]
    highs = np.where(tok >= HALF)[0]
    perm = np.concatenate([lows, highs])      # slot -> original position
    ptok = tok[perm]
    nlow = len(lows)
    gathers, stitches = _plan_gathers(ptok, nlow)

    gidx_np = np.concatenate([_wrap_idxs(g[3]) for g in gathers], axis=1)
    tv = perm.astype(np.float32)              # original time per slot
    # msk[m][p, t]  = 1.0 iff time(128m+p) < time(t)   (L in [s, t] layout)
    # msk2[m][p, s] = 1.0 iff time(s) < time(128m+p)   (L^T in [t, s] layout)
    msk_np = np.concatenate(
        [(tv[None, :] > tv[128 * m: 128 * (m + 1), None]).astype(np.float32)
         for m in range(TCH)], axis=1)        # [128, TCH*T]
    msk2_np = np.concatenate(
        [(tv[None, :] < tv[128 * m: 128 * (m + 1), None]).astype(np.float32)
         for m in range(TCH)], axis=1)        # [128, TCH*T]

    nc = bacc.Bacc("TRN2", target_bir_lowering=False, debug=False,
                   enable_asserts=False, num_devices=1)

    proj_d = nc.dram_tensor("proj", [VOCAB, N], dt.float32, kind="ExternalInput")
    gidx_d = nc.dram_tensor("gidx", list(gidx_np.shape), dt.int16, kind="ExternalInput")
    msk_d = nc.dram_tensor("msk", [128, TCH * T], dt.float32, kind="ExternalInput")
    msk2_d = nc.dram_tensor("msk2", [128, TCH * T], dt.float32, kind="ExternalInput")
    out_d = nc.dram_tensor("tens", [128, TCH], dt.float32, kind="ExternalOutput")

    with tile.TileContext(nc) as tc, ExitStack() as ctx:
        pool = ctx.enter_context(tc.tile_pool(name="main", bufs=1))
        ppt = ctx.enter_context(tc.tile_pool(name="ppt", bufs=4, space="PSUM"))
        pacc = ctx.enter_context(tc.tile_pool(name="pacc", bufs=1, space="PSUM"))

        raw = pool.tile([128, TCH * N], dt.float32, tag="raw")
        scr = pool.tile([128, N], dt.float32, tag="scr")
        gidx = pool.tile([128, gidx_np.shape[1]], dt.int16, tag="gidx")
        msk = pool.tile([128, TCH * T], dt.float32, tag="msk")
        msk2 = pool.tile([128, TCH * T], dt.float32, tag="msk2")
        seg_topk = nseg > 0
        cand = pool.tile([128, 8 * max(nseg, 1) * TCH], dt.float32, tag="cand")
        rawc = None if seg_topk else pool.tile([128, TCH * N], dt.float32, tag="rawc")
        m8 = pool.tile([128, 24 * TCH], dt.float32, tag="m8")
        acts = pool.tile([128, TCH * N], dt.bfloat16, tag="acts")
        ident = pool.tile([128, 128], dt.bfloat16, tag="ident")
        xt = pool.tile([128, NCH * T], dt.bfloat16, tag="xt")
        gb = pool.tile([128, TCH * T], dt.bfloat16, tag="gb")
        lt = pool.tile([128, TCH * T], dt.float32, tag="lt")
        lb = pool.tile([128, TCH * T], dt.bfloat16, tag="lb")
        dump = pool.tile([128, T], dt.float32, tag="dump")
        prod1 = pool.tile([128, TCH * T], dt.float32, tag="prod1")
        prod2 = pool.tile([128, TCH * T], dt.float32, tag="prod2")
        cnt_pm = pool.tile([128, TCH], dt.float32, tag="cnt_pm")
        dotv = pool.tile([128, TCH], dt.float32, tag="dotv")
        pn2v = pool.tile([128, TCH], dt.float32, tag="pn2v")
        q_v = pool.tile([128, TCH], dt.float32, tag="q_v")
        r_v = pool.tile([128, TCH], dt.float32, tag="r_v")
        rec_v = pool.tile([128, TCH], dt.float32, tag="rec_v")
        prod_v = pool.tile([128, TCH], dt.float32, tag="prod_v")
        tens_v = pool.tile([128, TCH], dt.float32, tag="tens_v")
        pre_v = pool.tile([128, 1], dt.float32, tag="pre_v")

        # --- constants, ACT table preloads, small input DMAs ---
        nc.sync.dma_start(gidx[:], gidx_d.ap())
        nc.sync.dma_start(msk[:], msk_d.ap())
        nc.sync.dma_start(msk2[:], msk2_d.ap())
        # preload ACT function tables off the critical path (sqrt(1)=1)
        nc.gpsimd.memset(pre_v[:], 1.0)
        nc.scalar.activation(pre_v[:], pre_v[:], Act.Copy)
        nc.scalar.activation(pre_v[:], pre_v[:], Act.Sqrt)
        masks.make_identity(nc, ident[:])

        # --- 1. gathers (all parallel; disjoint dest tiles) + stitch ---
        raw3 = raw[:].rearrange("p (c n) -> p c n", n=N)
        scr3 = scr[:].rearrange("p (c n) -> p c n", n=N)
        proj_ap = proj_d.ap()
        for g, (dest, c, half, idxs) in enumerate(gathers):
            out_ap = raw3[:, c: c + 1, :] if dest == "raw" else scr3[:, 0:1, :]
            nc.gpsimd.dma_gather(
                out_ap=out_ap,
                in_ap=proj_ap[HALF:, :] if half else proj_ap,
                idxs_ap=gidx[:, 8 * g: 8 * g + (len(idxs) + 15) // 16],
                num_idxs=len(idxs),
                num_idxs_reg=int(len(idxs)),
                elem_size=N,
            )
        for c, off, rows in stitches:
            nc.scalar.activation(
                raw[off:off + rows, c * N:(c + 1) * N],
                scr[off:off + rows, :], Act.Copy)

        # --- 2+3. per token-chunk: top-20 threshold, acts (+ row counts) ---
        prev_last = None
        for c in range(TCH):
            rc = raw[:, c * N:(c + 1) * N]
            chunk_ops = []
            if seg_topk:
                segw = N // nseg
                cd = cand[:, c * 8 * nseg:(c + 1) * 8 * nseg]
                for s in range(nseg):
                    op = nc.vector.max(
                        cd[:, s * 8:(s + 1) * 8],
                        rc[:, s * segw:(s + 1) * segw])
                    chunk_ops.append(op)
                sel = cd
            else:
                op = nc.scalar.activation(rawc[:, c * N:(c + 1) * N], rc, Act.Copy)
                sel = rawc[:, c * N:(c + 1) * N]
                rc = sel
                chunk_ops.append(op)
            m1 = m8[:, c * 24 + 0: c * 24 + 8]
            m2 = m8[:, c * 24 + 8: c * 24 + 16]
            m3 = m8[:, c * 24 + 16: c * 24 + 24]
            src = sel if seg_topk else raw[:, c * N:(c + 1) * N]
            chunk_ops.append(nc.vector.max(m1, src))
            chunk_ops.append(nc.vector.match_replace(src, m1, src, -1e30))
            chunk_ops.append(nc.vector.max(m2, src))
            chunk_ops.append(nc.vector.match_replace(src, m2, src, -1e30))
            chunk_ops.append(nc.vector.max(m3, src))
            thr = m8[:, c * 24 + 19: c * 24 + 20]   # 4th of round 3 = 20th
            last = nc.vector.tensor_scalar(
                acts[:, c * N:(c + 1) * N], rc, thr, None, Alu.is_ge,
                Alu.add, accum_out=cnt_pm[:, c: c + 1])
            chunk_ops.append(last)
            # keep the DVE chain chunk-ordered so chunk 0 finishes early and
            # its transposes/G overlap chunk 1's top-k
            if prev_last is not None:
                for op in chunk_ops:
                    add_dep_helper(op.ins, prev_last.ins, sync=False,
                                   reason="chunk-order DVE chain")
            prev_last = last

        # --- 3b. PE transpose acts -> XT [neuron, token] (bf16) ---
        # blocks grouped by token-half r so all r=0 work (transpose, copy,
        # and the G half-matmuls below) overlaps chunk 1's top-k; four
        # 128x128 transposes pack into one PSUM tile so one wide copy
        # evacuates them. xt free layout: index = r*N + cn*128.
        for r in range(TCH):
            for g in range(NCH // 4):
                pt = ppt.tile([128, 512], dt.bfloat16, tag="pt")
                for j in range(4):
                    cn = g * 4 + j
                    nc.tensor.transpose(
                        pt[:, j * 128:(j + 1) * 128],
                        acts[:, r * N + cn * 128: r * N + (cn + 1) * 128],
                        ident[:],
                    )
                dst = xt[:, r * N + g * 512: r * N + (g + 1) * 512]
                if r == 0 or g % 2 == 0:
                    # ACT: the DVE must not be interrupted mid-top-k (r=0)
                    nc.scalar.activation(dst, pt[:], Act.Copy)
                else:
                    nc.vector.tensor_copy(dst, pt[:])

        # --- 4. G = X X^T  [T, T] f32 psum, via bf16 matmuls (exact),
        #        split by token-half r so the r=0 half runs early ---
        gps = []
        for m in range(TCH):
            gp = pacc.tile([128, T], dt.float32, tag=f"g{m}")
            gps.append(gp)
        # m-outer: gps[0] completes first so the DVE's masked muls (below)
        # start while gps[1]'s groups are still on the PE
        for m in range(TCH):
            for r in range(TCH):
                for cn in range(NCH):
                    nc.tensor.matmul(
                        gps[m][:, r * 128:(r + 1) * 128],
                        xt[:, m * N + cn * 128: m * N + (cn + 1) * 128],
                        xt[:, r * N + cn * 128: r * N + (cn + 1) * 128],
                        start=(cn == 0), stop=(cn == NCH - 1),
                    )

        # --- 5+6. masked prefix matrices straight from PSUM, M^T = L^T G,
        #        and the dot/pn2 row reductions — all split by token-half so
        #        every piece gated only on r=0 data runs during chunk 1's
        #        top-k. Emission order == dependency order (r ascending).
        #        lb = bf16(G * msk)   (L, [s, t] layout — lhsT for M^T)
        #        lt = f32 (G * msk2)  (L^T, [t, s] layout — for row TTRs)
        #        gb = bf16(G)         (rhs for M^T) ---
        mts = []
        for m in range(TCH):
            mt = pacc.tile([128, T], dt.float32, tag=f"mt{m}")
            mts.append(mt)

        # gb halves by r (ACT — free during chunk 1's top-k); lb/lt as full
        # DVE ops (DVE is the serial resource; splitting only adds overhead)
        for r in range(TCH):
            for m in range(TCH):
                sl = slice(m * T + r * 128, m * T + (r + 1) * 128)
                nc.scalar.activation(gb[:, sl],
                                     gps[m][:, r * 128:(r + 1) * 128], Act.Copy)
        # NOTE: tensor_tensor_reduce is rejected by this runtime (device
        # NRT_EXEC_UNIT_UNRECOVERABLE) — reductions use an exact DVE product
        # followed by an ACT Copy with accum_out (HW-verified) instead.
        # Per-block interleave: all m=0 work is emitted before anything
        # gated on gps[1], so the DVE isn't head-of-line blocked.
        for m in range(TCH):
            ltm = lt[:, m * T:(m + 1) * T]
            nc.vector.tensor_mul(lb[:, m * T:(m + 1) * T], gps[m][:],
                                 msk[:, m * T:(m + 1) * T])
            nc.vector.tensor_mul(ltm, gps[m][:], msk2[:, m * T:(m + 1) * T])
            # dot[t] = sum_s L^T[t,s]^2 — off the critical DVE sequence
            # (GPSIMD product; dot only gates the final subtract)
            nc.gpsimd.tensor_mul(prod1[:, m * T:(m + 1) * T], ltm, ltm)
            nc.scalar.activation(dump[:], prod1[:, m * T:(m + 1) * T],
                                 Act.Copy, accum_out=dotv[:, m: m + 1])
        for m in range(TCH):
            for b in range(TCH):
                nc.tensor.matmul(
                    mts[m][:],
                    lb[:, b * T + m * 128: b * T + (m + 1) * 128],
                    gb[:, b * T:(b + 1) * T],
                    start=(b == 0), stop=(b == TCH - 1),
                )
            nc.vector.tensor_mul(prod2[:, m * T:(m + 1) * T],
                                 lt[:, m * T:(m + 1) * T], mts[m][:])
            # scale = cnt folds q = pn2*cnt into the accumulate (exact: all
            # terms are integers < 2^24), so sqrt follows directly on ACT
            nc.scalar.activation(dump[:], prod2[:, m * T:(m + 1) * T],
                                 Act.Copy, scale=cnt_pm[:, m: m + 1],
                                 accum_out=q_v[:, m: m + 1])

        # --- 7. final per-token math on [128, TCH] (token-major):
        #     tension = 1 - dot/denom = (denom - dot)/denom,
        #     denom = sqrt(pn2*cnt) + 1e-6; q = pn2*cnt from the accum above.
        # Split per token-block column: block 0's chain runs while block 1's
        # pn2 accumulate is still in flight. ---
        for m in range(TCH):
            sl = slice(m, m + 1)
            nc.scalar.activation(r_v[:, sl], q_v[:, sl], Act.Sqrt)
            nc.vector.tensor_scalar_add(r_v[:, sl], r_v[:, sl], 1e-6)
            nc.vector.tensor_tensor(prod_v[:, sl], r_v[:, sl], dotv[:, sl],
                                    Alu.subtract)
            nc.vector.reciprocal(rec_v[:, sl], r_v[:, sl])
            nc.vector.tensor_mul(tens_v[:, sl], prod_v[:, sl], rec_v[:, sl])

        # --- 8. output: plain [128, TCH] DMA; host maps (p, c) -> t = 128c+p ---
        nc.sync.dma_start(out_d.ap(), tens_v[:])

    nc.compile()

    in_map = {
        "proj": None,  # filled by caller (f32 [VOCAB, N])
        "gidx": gidx_np,
        "msk": msk_np,
        "msk2": msk2_np,
    }
    return nc, in_map, perm


def _check_input(projection, sigma, tokens):
    """Host-side guards. Returns (fast_ok, nseg):
    fast_ok — the algebraic rewrite is exact (sigma==0, clip never binds);
    nseg    — widest valid segmentation for the segmented top-k (a
    segmentation is valid when taking the top-8 of every segment still
    captures all of each row's top-20 values), or 0 for the full-row path."""
    if np.any(np.asarray(sigma)):
        return False, 0
    proj = np.asarray(projection, np.float32)
    raw = proj[np.asarray(tokens)]
    kth = np.partition(raw, N - K, axis=1)[:, N - K]
    acts = (raw >= kth[:, None]).astype(np.float32)
    coact = acts.T @ acts
    fast_ok = float(coact.max()) <= 100.0
    nseg = 0
    for cand_nseg in (8, 16):
        segs = raw.reshape(T, cand_nseg, N // cand_nseg)
        cand = -np.sort(-segs, axis=2)[:, :, :8].reshape(T, cand_nseg * 8)
        thr_dev = -np.sort(-cand, axis=1)[:, K - 1]
        if bool(np.all(thr_dev == kth)):
            nseg = cand_nseg
            break
    return fast_ok, nseg


def kernel(projection, sigma, tokens, plasticity):
    global LAST_RESULT
    projection = np.ascontiguousarray(np.asarray(projection, np.float32))
    sigma = np.asarray(sigma, np.float32)
    tokens = np.asarray(tokens).astype(np.int64)
    plast = int(np.asarray(plasticity).reshape(-1)[0]) if np.ndim(plasticity) else int(plasticity)

    if not plast:
        # sigma never updates; with sigma == 0, pred == 0 -> tension == 1.
        if not np.any(sigma):
            return np.ones(T, np.float32)
        return _numpy_fallback(projection, sigma, tokens, plast)
    fast_ok, nseg = _check_input(projection, sigma, tokens)
    if not fast_ok:
        return _numpy_fallback(projection, sigma, tokens, plast)

    from concourse.bass_utils import run_bass_kernel_spmd

    nc, in_map, perm = _build(tokens, nseg=nseg)
    in_map["proj"] = projection
    n_cores = int(os.environ.get("BDH_CORES", "8"))
    try:
        res = run_bass_kernel_spmd(
            nc,
            [dict(in_map) for _ in range(n_cores)],
            core_ids=list(range(n_cores)),
        )
    except ModuleNotFoundError:
        # BASS_TRACE was requested but this axon build has no NTFF hook.
        os.environ["BASS_NEVER_TRACE"] = "1"
        res = run_bass_kernel_spmd(
            nc,
            [dict(in_map) for _ in range(n_cores)],
            core_ids=list(range(n_cores)),
        )
    LAST_RESULT = res
    # device layout [p, c] -> slot t = 128c + p; then slot -> original time
    tens_slots = res.results[0]["tens"].reshape(128, TCH).T.reshape(T)
    out = np.empty(T, np.float32)
    out[perm] = tens_slots.astype(np.float32)
    return out
